# revision 1
# baseline (speedup 1.0000x reference)
"""MixMamba Trainium2 Bass kernel (8-core SPMD).

Sharding:
 - Mamba branch: data-parallel, core = batch element (8 batches, 8 cores).
 - Local conv branch: channel-parallel (24 of 192 channels per core, all
   batches) because training-mode BatchNorm needs cross-batch stats; the
   host sums the per-core partial pw2 outputs during unshard.

Mamba selective scan: for each (direction, d-tile, state n) the recurrence
h_t = exp(-n*delta_t)*h_{t-1} + B_t[n]*delta_t*u_t runs as one hardware
`tensor_tensor_scan` over 1025 elements (1024 steps + 1 zero "reset" pad
column so consecutive uses never leak state).  The decay tensor is built
on the Scalar engine as Exp(scale*delta) with scale=-n (free affine), the
B/C rows are partition-broadcast by DMA, and y = sum_n C_n*h_n accumulates
through a balanced binary tree of bf16 adds on the Vector engine.
"""
import os
import sys
import numpy as np

for _p in ("/opt/trn_rl_repo",):
    if _p not in sys.path and os.path.isdir(_p):
        sys.path.insert(0, _p)

import ml_dtypes

bf16 = ml_dtypes.bfloat16

B, Hh, Ww = 8, 32, 32
L = 1024
D_MODEL = 384
MD = 192
DI = 384
NS = 16
DC = 4
DTR = 12
NDIR = 4
RED = 48
IC = 192
KC = 8
NCORES = 8
CSL = IC // NCORES
DT3 = 3
LP = L + 1
HF = L // 512  # matmul column halves

# leaf write targets / post-add chains for the 16-leaf balanced sum
# part indices: 0..4 ; leaf n written to _TGT[n]; then p_dst += p_src chains
_TGT = {1: 1, 2: 0, 3: 2, 4: 0, 5: 2, 6: 0, 7: 3, 8: 0,
        9: 2, 10: 0, 11: 3, 12: 0, 13: 3, 14: 0, 15: 4, 16: 0}
_POST = {2: [(1, 0)], 4: [(2, 0), (1, 2)], 6: [(2, 0)],
         8: [(3, 0), (2, 3), (1, 2)], 10: [(2, 0)], 12: [(3, 0), (2, 3)],
         14: [(3, 0)], 16: [(4, 0), (3, 4), (2, 3), (1, 2)]}
# final sum lands in part[1]


def build_nc():
    import os as _os
    SKIP_LOCAL = _os.environ.get("K_SKIP_LOCAL") == "1"
    SKIP_SCAN = _os.environ.get("K_SKIP_SCAN") == "1"
    SKIP_BIATTN = _os.environ.get("K_SKIP_BIATTN") == "1"
    NDIRS = int(_os.environ.get("K_NDIRS", NDIR))
    from concourse import bacc, tile, mybir

    f32 = mybir.dt.float32
    bfl = mybir.dt.bfloat16
    AF = mybir.ActivationFunctionType
    OP = mybir.AluOpType
    X = mybir.AxisListType.X

    nc = bacc.Bacc("TRN2", target_bir_lowering=False, debug=False,
                   num_devices=NCORES)

    def din(name, shape, dt=bfl):
        return nc.dram_tensor(name, list(shape), dt, kind="ExternalInput")

    d_xm = din("xm", (2, 128, L))
    d_winp = din("w_inproj", (2, 128, 2 * DI))
    d_cvd = din("conv_diag", (NDIR, 128, DC, DT3, 128))
    d_convb = din("conv_bias", (128, NDIR, DT3), f32)
    d_wxp = din("w_xproj", (128, NDIR, DT3, DTR + 2 * NS))
    d_wdt = din("w_dtproj", (DTR, NDIR, DI))
    d_dtb = din("dt_bias", (128, NDIR, DT3), f32)
    d_dp = din("dp", (128, NDIR, DT3), f32)
    d_wout = din("w_outproj", (128, DT3, MD))
    d_lng = din("ln_g_div", (128, DT3), f32)
    d_lnb = din("ln_b", (128, DT3), f32)
    d_wgr = din("w_gr", (128, DT3, RED))
    d_grb = din("gr_b", (RED, 1), f32)
    d_wcs = din("w_cs", (RED, DI))
    d_csb = din("cs_b", (128, DT3), f32)
    d_ones = din("ones_col", (128, 1))
    d_xloc = din("xloc", (2, 128, B, L))
    d_wpw1 = din("w_pw1", (2, 128, 64))
    d_p1ba = din("pw1_b_a", (CSL, 1), f32)
    d_p1bg = din("pw1_b_g", (CSL, 1), f32)
    d_dwd = din("dw_diag", (128, KC, 2, 128))
    d_dwb = din("dw_bias", (128, 2), f32)
    d_bng = din("bn_g", (CSL, 1), f32)
    d_bnb = din("bn_b", (CSL, 1), f32)
    d_bsum = din("bsum_mat", (2, 128, CSL), f32)
    d_wpw2 = din("w_pw2", (CSL, IC))
    d_p2b = din("pw2_bias", (128, 2), f32)

    d_out_m = nc.dram_tensor("out_mamba", [2, 128, L], f32,
                             kind="ExternalOutput")
    d_out_l = nc.dram_tensor("out_local", [B, 2, 128, L], f32,
                             kind="ExternalOutput")

    with tile.TileContext(nc) as tc, \
            tc.tile_pool(name="w", bufs=1) as wp, \
            tc.tile_pool(name="pers", bufs=1) as pp, \
            tc.tile_pool(name="tr1", bufs=1) as tr1, \
            tc.tile_pool(name="tr2", bufs=2) as tr2, \
            tc.tile_pool(name="dr", bufs=2, space="DRAM") as dr, \
            tc.tile_pool(name="ps", bufs=2, space="PSUM") as ps, \
            tc.tile_pool(name="psm", bufs=1, space="PSUM") as psm:

        import itertools as _it
        _cnt = _it.count()

        def mktile(pool, shape, dt, tag):
            return pool.tile(list(shape), dt, tag=tag,
                             name=f"{tag.replace(' ', '')}_{next(_cnt)}")

        def load(dram, shape, dt=bfl, tag=None, pool=wp):
            t = mktile(pool, shape, dt, tag)
            nc.sync.dma_start(t[:], dram[:] if not isinstance(dram, tuple)
                              else dram[0])
            return t

        s_xm = [mktile(wp, (128, L), bfl, f"xm{i}") for i in range(2)]
        s_winp = [mktile(wp, (128, 2 * DI), bfl, f"wi{i}") for i in range(2)]
        for i in range(2):
            nc.sync.dma_start(s_xm[i][:], d_xm[i])
            nc.sync.dma_start(s_winp[i][:], d_winp[i])
        s_wxp = load(d_wxp, (128, NDIR, DT3, DTR + 2 * NS), tag="wxp")
        s_wdt = load(d_wdt, (DTR, NDIR, DI), tag="wdt")
        s_convb = load(d_convb, (128, NDIR, DT3), f32, tag="convb")
        s_dtb = load(d_dtb, (128, NDIR, DT3), f32, tag="dtb")
        s_dp = load(d_dp, (128, NDIR, DT3), f32, tag="dp")
        s_wout = load(d_wout, (128, DT3, MD), tag="wout")
        s_lng = load(d_lng, (128, DT3), f32, tag="lng")
        s_lnb = load(d_lnb, (128, DT3), f32, tag="lnb")
        s_wgr = load(d_wgr, (128, DT3, RED), tag="wgr")
        s_grb = load(d_grb, (RED, 1), f32, tag="grb")
        s_wcs = load(d_wcs, (RED, DI), tag="wcs")
        s_csb = load(d_csb, (128, DT3), f32, tag="csb")
        s_ones = load(d_ones, (128, 1), tag="ones")
        s_wpw1 = [mktile(wp, (128, 64), bfl, f"w1{i}") for i in range(2)]
        s_bsum = [mktile(wp, (128, CSL), f32, f"bs{i}") for i in range(2)]
        for i in range(2):
            nc.sync.dma_start(s_wpw1[i][:], d_wpw1[i])
            nc.sync.dma_start(s_bsum[i][:], d_bsum[i])
        s_p1ba = load(d_p1ba, (CSL, 1), f32, tag="p1ba")
        s_p1bg = load(d_p1bg, (CSL, 1), f32, tag="p1bg")
        s_dwd = load(d_dwd, (128, KC, 2, 128), tag="dwd")
        s_dwb = load(d_dwb, (128, 2), f32, tag="dwb")
        s_bng = load(d_bng, (CSL, 1), f32, tag="bng")
        s_bnb = load(d_bnb, (CSL, 1), f32, tag="bnb")
        s_wpw2 = load(d_wpw2, (CSL, IC), tag="wpw2")
        s_p2b = load(d_p2b, (128, 2), f32, tag="p2b")

        epsb = mktile(pp, (128, 1), f32, "epsb")
        nc.gpsimd.memset(epsb[:], 1e-5)

        def mm(out_ap, lhsT, rhs_ap, start, stop, cols=L):
            """matmul split into <=512-column pieces."""
            nh = (cols + 511) // 512
            for h in range(nh):
                c0, c1 = h * 512, min((h + 1) * 512, cols)
                nc.tensor.matmul(out_ap[:, c0:c1], lhsT, rhs_ap[:, c0:c1],
                                 start=start, stop=stop)

        if not SKIP_LOCAL:
            # ======== local branch: pw1 + GLU ========
            hglu = [mktile(pp, (128, L), bfl, f"hglu{i}") for i in range(2)]
            for pt in range(2):
                nc.gpsimd.memset(hglu[pt][:], 0.0)
            for b in range(B):
                pt, row = b // 4, (b % 4) * 32
                p1 = mktile(psm, (64, L), f32, "pml")
                for kt in range(2):
                    xlb = mktile(tr1, (128, L), bfl, "xlb")
                    nc.sync.dma_start(xlb[:], d_xloc[kt, :, b, :])
                    mm(p1[:], s_wpw1[kt][:], xlb[:],
                       start=(kt == 0), stop=(kt == 1))
                sig = mktile(tr1, (CSL, L), bfl, "lsig")
                nc.scalar.activation(sig[:], p1[32:32 + CSL, :], AF.Sigmoid,
                                     bias=s_p1bg[:])
                av = mktile(tr1, (CSL, L), bfl, "lav")
                nc.scalar.activation(av[:], p1[0:CSL, :], AF.Identity,
                                     bias=s_p1ba[:])
                nc.vector.tensor_mul(hglu[pt][row:row + CSL, :], av[:], sig[:])

            # ================= local: depthwise conv + BN stats ========
            hconv = [mktile(pp, (128, L), bfl, f"hconv{i}") for i in range(2)]
            lsum = mktile(pp, (128, 2, 2), f32, "lsum")  # [:, pt, 0]=sum 1=sumsq
            for pt in range(2):
                pc = mktile(psm, (128, L), f32, "pml")
                order = [4, 0, 1, 2, 3, 5, 6, 7]  # shift-0 tap first (start)
                for j, k in enumerate(order):
                    sh = 4 - k
                    o0, i0 = max(0, sh), max(0, -sh)
                    ln = L - abs(sh)
                    for h in range(HF):
                        a0 = max(o0, h * 512)
                        a1 = min(o0 + ln, (h + 1) * 512)
                        if a1 <= a0:
                            continue
                        nc.tensor.matmul(
                            pc[:, a0:a1], s_dwd[:, k, pt, :],
                            hglu[pt][:, a0 - sh:a1 - sh],
                            start=(j == 0), stop=(j == len(order) - 1),
                            skip_group_check=True)
                nc.scalar.activation(hconv[pt][:], pc[:], AF.Identity,
                                     bias=s_dwb[:, pt:pt + 1],
                                     accum_out=lsum[:, pt, 0:1])
                sqd = mktile(tr1, (128, L), bfl, "lsq")
                nc.scalar.activation(sqd[:], hconv[pt][:], AF.Square,
                                     accum_out=lsum[:, pt, 1:2])
            pbn = mktile(psm, (CSL, 2), f32, "psms")
            for kt in range(2):
                nc.tensor.matmul(pbn[:], s_bsum[kt][:], lsum[:, kt, :],
                                 start=(kt == 0), stop=(kt == 1))
            st24 = mktile(pp, (CSL, 6), f32, "st24")  # mu E2 musq var rstd -
            nc.scalar.activation(st24[:, 0:1], pbn[:, 0:1], AF.Copy,
                                 scale=1.0 / (B * L))
            nc.scalar.activation(st24[:, 1:2], pbn[:, 1:2], AF.Copy,
                                 scale=1.0 / (B * L))
            nc.scalar.activation(st24[:, 2:3], st24[:, 0:1], AF.Square)
            nc.vector.tensor_sub(st24[:, 3:4], st24[:, 1:2], st24[:, 2:3])
            nc.scalar.activation(st24[:, 4:5], st24[:, 3:4],
                                 AF.Abs_reciprocal_sqrt, bias=epsb[0:CSL, :])
            sc24 = mktile(pp, (CSL, 2), f32, "sc24")  # scale, bias
            nc.vector.tensor_mul(sc24[:, 0:1], st24[:, 4:5], s_bng[:])
            nc.vector.tensor_mul(sc24[:, 1:2], st24[:, 0:1], sc24[:, 0:1])
            nc.vector.tensor_sub(sc24[:, 1:2], s_bnb[:], sc24[:, 1:2])
            screp = mktile(pp, (128, 2, 2), f32, "screp")  # [:, pt, (scale,bias)]
            nc.gpsimd.memset(screp[:], 0.0)
            for pt in range(2):
                for bb in range(4):
                    nc.sync.dma_start(screp[bb * 32:bb * 32 + CSL, pt, :],
                                      sc24[:, :])

        # ================= mamba: in_proj =================
        xz = [mktile(pp, (128, L), bfl, f"xz{m}") for m in range(6)]
        for m in range(6):
            pxz = mktile(ps, (128, L), f32, "pmm")
            for kt in range(2):
                mm(pxz[:], s_winp[kt][:, m * 128:(m + 1) * 128], s_xm[kt][:],
                   start=(kt == 0), stop=(kt == 1))
            nc.scalar.activation(xz[m][:], pxz[:], AF.Copy)

        # ================= mamba per-direction =================
        acc = mktile(pp, (128, DT3, L), f32, "acc")
        nc.gpsimd.memset(acc[:], 0.0)

        for d in range(NDIRS):
            if d == 3 and not SKIP_LOCAL:
                hbns = [mktile(pp, (128, L), bfl, f"hbns{i}") for i in range(2)]
                for pt in range(2):
                    nc.scalar.activation(hbns[pt][:], hconv[pt][:], AF.Silu,
                                         scale=screp[:, pt, 0:1],
                                         bias=screp[:, pt, 1:2])
                for b in range(B):
                    pt, row = b // 4, (b % 4) * 32
                    stage = mktile(tr1, (CSL, L), bfl, "stage")
                    nc.vector.tensor_copy(stage[:], hbns[pt][row:row + CSL, :])
                    for mt, (c0, mw) in enumerate(((0, 128), (128, IC - 128))):
                        pw = mktile(psm, (128, L), f32, "pml")
                        mm(pw[0:mw, :], s_wpw2[:, c0:c0 + mw], stage[:],
                           start=True, stop=True)
                        ev = mktile(tr1, (128, L), f32, "lev")
                        nc.scalar.activation(ev[0:mw, :], pw[0:mw, :], AF.Identity,
                                             bias=s_p2b[0:mw, mt:mt + 1])
                        nc.sync.dma_start(d_out_l[b, mt, 0:mw, :], ev[0:mw, :])


            cvd_t = mktile(tr1, (128, DC, DT3, 128), bfl, "cvd")
            nc.sync.dma_start(cvd_t[:], d_cvd[d])
            # --- direction-ordered xz (materialize for dirs 1..3) ---
            if d == 0:
                xzd = xz
                xv = [t[:] for t in xz]
            else:
                xzd_t = mktile(tr2, (128, 6, L), bfl, "xzd")
                for m in range(6):
                    if d == 1:
                        nc.vector.tensor_copy(xzd_t[:, m, :], xz[m][:, ::-1])
                    elif d == 2:
                        nc.vector.tensor_copy(
                            xzd_t[:, m, :].rearrange("p (w h) -> p w h", w=Ww),
                            xz[m][:].rearrange("p (h w) -> p h w", h=Hh)
                            .transpose([0, 2, 1]))
                    else:
                        nc.vector.tensor_copy(
                            xzd_t[:, m, :],
                            xzd_prev[:, m, ::-1])  # noqa: F821
                xv = [xzd_t[:, m, :] for m in range(6)]
                if d == 2:
                    xzd_prev = xzd_t
            # --- conv + silu -> u ; silu(z) -> sz ---
            u_t = mktile(tr2, (128, DT3, L), bfl, "u")
            du_t = mktile(tr2, (128, DT3, L), bfl, "du")
            sz_t = mktile(tr2, (128, DT3, L), bfl, "sz")
            dl_t = mktile(tr2, (128, DT3, L), bfl, "delta")
            for j in range(DT3):
                pu = mktile(ps, (128, L), f32, "pmm")
                order = [3, 0, 1, 2]
                for jj, k in enumerate(order):
                    sh = 3 - k
                    for h in range(HF):
                        a0 = max(sh, h * 512)
                        a1 = (h + 1) * 512
                        if a1 <= a0:
                            continue
                        nc.tensor.matmul(
                            pu[:, a0:a1], cvd_t[:, k, j, :],
                            xv[j][:, a0 - sh:a1 - sh],
                            start=(jj == 0), stop=(jj == len(order) - 1),
                            skip_group_check=True)
                nc.scalar.activation(u_t[:, j, :], pu[:], AF.Silu,
                                     bias=s_convb[:, d, j:j + 1])
                nc.scalar.activation(sz_t[:, j, :], xv[j + 3], AF.Silu)
            # --- x_proj ---
            pdbl = mktile(ps, (DTR + 2 * NS, L), f32, "pmm")
            for j in range(DT3):
                mm(pdbl[:], s_wxp[:, d, j, :], u_t[:, j, :],
                   start=(j == 0), stop=(j == DT3 - 1))
            dbl = mktile(tr1, (DTR + 2 * NS, L), bfl, "dbl")
            nc.scalar.activation(dbl[:], pdbl[:], AF.Copy)
            ddbl = mktile(dr, (DTR + 2 * NS, L), bfl, "ddbl")
            nc.sync.dma_start(ddbl[:], dbl[:])
            # --- delta / du ---
            for j in range(DT3):
                pdp = mktile(ps, (128, L), f32, "pmm")
                mm(pdp[:], s_wdt[:, d, j * 128:(j + 1) * 128], dbl[0:DTR, :],
                   start=True, stop=True)
                # softplus(x) = ln(1 + exp(x)); Exp and Ln share a table set
                spt = mktile(tr1, (128, L), bfl, "spt")
                nc.scalar.activation(spt[:], pdp[:], AF.Exp,
                                     bias=s_dtb[:, d, j:j + 1])
                nc.scalar.activation(dl_t[:, j, :], spt[:], AF.Ln, bias=1.0)
            nc.vector.tensor_mul(du_t[:], dl_t[:], u_t[:])

            if not SKIP_SCAN:
                # --- scan ---
                got = mktile(tr1, (128, DT3, L), bfl, "got")
                for j in range(DT3):
                    a_t = [mktile(pp, (128, LP), f32, f"a{i}") for i in range(2)]
                    b_t = mktile(pp, (128, LP), bfl, "b0")
                    h_t = mktile(pp, (128, LP), bfl, "h0")
                    for t_ in a_t:
                        nc.gpsimd.memset(t_[:, L:LP], 0.0)
                    nc.gpsimd.memset(b_t[:, L:LP], 0.0)
                    part = [mktile(pp, (128, L), bfl, f"p{lv}") for lv in range(5)]
                    for n in range(1, NS + 1):
                        sl = (n - 1) % 2
                        brep = mktile(pp, (128, L), bfl, f"br{sl}")
                        crep = mktile(pp, (128, L), bfl, f"cr{sl}")
                        nc.scalar.activation(a_t[sl][:, 0:L], dl_t[:, j, :],
                                             AF.Exp, scale=-float(n))
                        nc.sync.dma_start(
                            brep[:],
                            ddbl[DTR + n - 1:DTR + n, :].to_broadcast((128, L)))
                        nc.sync.dma_start(
                            crep[:],
                            ddbl[DTR + NS + n - 1:DTR + NS + n, :]
                            .to_broadcast((128, L)))
                        nc.vector.tensor_mul(b_t[:, 0:L], du_t[:, j, :], brep[:])
                        nc.vector.tensor_tensor_scan(h_t[:], a_t[sl][:], b_t[:],
                                                     0.0, OP.mult, OP.add)
                        nc.vector.tensor_mul(part[_TGT[n]][:], h_t[:, 0:L],
                                             crep[:])
                        for dst, src in _POST.get(n, ()):
                            nc.vector.tensor_add(part[dst][:], part[dst][:],
                                                 part[src][:])
                    # y = tree_sum + u*Dp  (into part[0])
                    nc.vector.scalar_tensor_tensor(
                        part[0][:], u_t[:, j, :], s_dp[:, d, j:j + 1],
                        part[1][:], OP.mult, OP.add)
                    # gate with silu(z), written back to original token order
                    if d == 0:
                        nc.vector.tensor_mul(got[:, j, :], part[0][:],
                                             sz_t[:, j, :])
                    elif d == 1:
                        nc.vector.tensor_mul(got[:, j, ::-1], part[0][:],
                                             sz_t[:, j, :])
                    else:
                        # contiguous gate mul in v-order, then one strided
                        # copy into token order (4.6us strided mul -> 1.7us)
                        gtmp = mktile(pp, (128, L), bfl, "gtmp")
                        if d == 2:
                            nc.vector.tensor_mul(gtmp[:], part[0][:],
                                                 sz_t[:, j, :])
                        else:
                            nc.vector.tensor_mul(gtmp[:], part[0][:, ::-1],
                                                 sz_t[:, j, ::-1])
                        gv = got[:, j, :] \
                            .rearrange("p (h w) -> p h w", h=Hh) \
                            .transpose([0, 2, 1])
                        nc.vector.tensor_copy(
                            gv, gtmp[:].rearrange("p (w h) -> p w h", w=Ww))

            if not SKIP_BIATTN:
                # --- biattn ---
                sq = mktile(tr1, (128, DT3, L), bfl, "bsqt")
                nc.scalar.activation(sq[:], got[:], AF.Square)
                dstat = mktile(dr, (2, L), bfl, "dstat")
                for src_t, outrow in ((got, 0), (sq, 1)):
                    rowb = mktile(pp, (1, L), bfl, "rowb")
                    for h in range(HF):
                        pbi = mktile(psm, (1, 512), f32, "psm")
                        for j in range(DT3):
                            nc.tensor.matmul(pbi[:], s_ones[:],
                                             src_t[:, j, h * 512:(h + 1) * 512],
                                             start=(j == 0), stop=(j == DT3 - 1))
                        nc.scalar.activation(rowb[:, h * 512:(h + 1) * 512],
                                             pbi[:], AF.Copy, scale=1.0 / DI)
                    nc.sync.dma_start(dstat[outrow:outrow + 1, :], rowb[:])
                # t-partitioned stats: (128, 8) with t = p*8 + f
                sumT = mktile(pp, (128, 2, 8), bfl, "sumT")
                nc.sync.dma_start(sumT[:],
                                  dstat[:].rearrange("r (p f) -> p r f", p=128))
                mu8 = mktile(pp, (128, 8), f32, "mu8")
                ms8 = mktile(pp, (128, 8), f32, "ms8")
                rstd8 = mktile(pp, (128, 8), f32, "rstd8")
                nc.vector.tensor_copy(mu8[:], sumT[:, 0, :])
                nc.scalar.activation(ms8[:], mu8[:], AF.Square)
                nc.vector.tensor_sub(ms8[:], sumT[:, 1, :], ms8[:])
                nc.scalar.activation(rstd8[:], ms8[:],
                                     AF.Abs_reciprocal_sqrt, bias=epsb[:])
                rstdb = mktile(pp, (128, 8), bfl, "rstdb")
                nc.vector.tensor_copy(rstdb[:], rstd8[:])
                drst = mktile(dr, (1, L), bfl, "drst")
                nc.sync.dma_start(
                    drst[:].rearrange("r (p f) -> p (r f)", p=128), rstdb[:])
                rsb = mktile(pp, (128, L), bfl, "rsb")
                nc.sync.dma_start(rsb[:], drst[:].to_broadcast((128, L)))
                # q2 = sum_t mu*rstd
                q2t8 = mktile(pp, (128, 8), bfl, "q2t8")
                nc.vector.tensor_mul(q2t8[:], mu8[:], rstd8[:])
                pq2 = mktile(psm, (1, 8), f32, "psms")
                nc.tensor.matmul(pq2[:], s_ones[:], q2t8[:], start=True, stop=True)
                q2row = mktile(pp, (1, 8), f32, "q2row")
                nc.scalar.activation(q2row[:], pq2[:], AF.Copy)
                q2 = mktile(pp, (1, 1), f32, "q2")
                nc.vector.reduce_sum(q2[:], q2row[:], axis=X)
                dq2 = mktile(dr, (1, 1), f32, "dq2")
                nc.sync.dma_start(dq2[:], q2[:])
                q2r = mktile(pp, (128, 1), f32, "q2r")
                nc.sync.dma_start(q2r[:], dq2[:].to_broadcast((128, 1)))
                t1 = mktile(tr1, (128, DT3, L), bfl, "bsqt")
                q1 = mktile(pp, (128, DT3), f32, "q1")
                gp = mktile(pp, (128, DT3), f32, "gp")
                gpb = mktile(pp, (128, DT3), bfl, "gpb")
                for j in range(DT3):
                    nc.vector.tensor_mul(t1[:, j, :], got[:, j, :], rsb[:])
                    nc.scalar.activation(t1[:, j, :], t1[:, j, :], AF.Copy,
                                         accum_out=q1[:, j:j + 1])
                    nc.vector.tensor_sub(gp[:, j:j + 1], q1[:, j:j + 1], q2r[:])
                    nc.vector.tensor_mul(gp[:, j:j + 1], gp[:, j:j + 1],
                                         s_lng[:, j:j + 1])
                    nc.vector.tensor_add(gp[:, j:j + 1], gp[:, j:j + 1],
                                         s_lnb[:, j:j + 1])
                nc.vector.tensor_copy(gpb[:], gp[:])
                pgr = mktile(psm, (RED, 1), f32, "psms")
                for j in range(DT3):
                    nc.tensor.matmul(pgr[:], s_wgr[:, j, :], gpb[:, j:j + 1],
                                     start=(j == 0), stop=(j == DT3 - 1))
                gg = mktile(pp, (RED, 4), f32, "gg")
                nc.scalar.activation(gg[:, 0:1], pgr[:], AF.Identity,
                                     bias=s_grb[:])
                nc.scalar.activation(gg[:, 1:2], gg[:, 0:1], AF.Erf,
                                     scale=0.7071067811865476)
                nc.vector.tensor_scalar_add(gg[:, 1:2], gg[:, 1:2], 1.0)
                nc.vector.tensor_mul(gg[:, 2:3], gg[:, 0:1], gg[:, 1:2])
                ggb = mktile(pp, (RED, 1), bfl, "ggb")
                nc.scalar.activation(ggb[:], gg[:, 2:3], AF.Copy, scale=0.5)
                cvec = mktile(pp, (128, DT3), f32, "cvec")
                for j in range(DT3):
                    pcs = mktile(psm, (128, 1), f32, "psms")
                    nc.tensor.matmul(pcs[:], s_wcs[:, j * 128:(j + 1) * 128],
                                     ggb[:], start=True, stop=True)
                    nc.scalar.activation(cvec[:, j:j + 1], pcs[:], AF.Sigmoid,
                                         bias=s_csb[:, j:j + 1])
                    nc.vector.scalar_tensor_tensor(
                        acc[:, j, :], got[:, j, :], cvec[:, j:j + 1],
                        acc[:, j, :], OP.mult, OP.add)

        # ---- out_proj ----
        accb = mktile(pp, (128, DT3, L), bfl, "accb")
        nc.vector.tensor_copy(accb[:], acc[:])
        for mt in range(2):
            mw = 128 if mt == 0 else MD - 128
            pmo = mktile(ps, (128, L), f32, "pmm")
            for j in range(DT3):
                mm(pmo[0:mw, :], s_wout[:, j, mt * 128:mt * 128 + mw],
                   accb[:, j, :], start=(j == 0), stop=(j == DT3 - 1))
            evm = mktile(tr1, (128, L), f32, "lev")
            nc.scalar.activation(evm[0:mw, :], pmo[0:mw, :], AF.Copy)
            nc.sync.dma_start(d_out_m[mt, 0:mw, :], evm[0:mw, :])

    nc.compile()
    return nc


# ------------------------------------------------------------- host side
def _pad_rows(a, rows):
    out = np.zeros((rows,) + a.shape[1:], a.dtype)
    out[:a.shape[0]] = a
    return out


def _bfl(a):
    return np.ascontiguousarray(a).astype(bf16)


def build_in_maps(inputs):
    f = lambda k: np.asarray(inputs[k], np.float32)
    x = f("x")
    ipw, conv_w, conv_b = f("in_proj_w"), f("conv_w"), f("conv_b")
    xproj_w, dtproj_w, dtproj_b = f("xproj_w"), f("dtproj_w"), f("dtproj_b")
    Dp, out_proj_w = f("Dp"), f("out_proj_w")
    ln_g, ln_b = f("attn_ln_g"), f("attn_ln_b")
    gr_w, gr_b = f("attn_gr_w"), f("attn_gr_b")
    cs_w, cs_b = f("attn_cs_w"), f("attn_cs_b")
    pw1_w, pw1_b = f("pw1_w"), f("pw1_b")
    dw_w, dw_b = f("dw_w"), f("dw_b")
    bn_g, bn_b = f("bn_g"), f("bn_b")
    pw2_w, pw2_b = f("pw2_w"), f("pw2_b")

    cvd = np.zeros((NDIR, 128, DC, DT3, 128), np.float32)
    r = np.arange(128)
    for d in range(NDIR):
        for k in range(DC):
            for j in range(DT3):
                cvd[d, r, k, j, r] = conv_w[d, j * 128:(j + 1) * 128, k]
    shared = {
        "w_inproj": _bfl(_pad_rows(ipw.T, 256).reshape(2, 128, 2 * DI)),
        "conv_diag": _bfl(cvd),
        "conv_bias": np.ascontiguousarray(
            conv_b.T.reshape(DT3, 128, NDIR).transpose(1, 2, 0)),
        "w_xproj": _bfl(xproj_w.transpose(2, 0, 1).reshape(
            DT3, 128, NDIR, DTR + 2 * NS).transpose(1, 2, 0, 3)),
        "w_dtproj": _bfl(dtproj_w.transpose(2, 0, 1)),
        "dt_bias": np.ascontiguousarray(
            dtproj_b.T.reshape(DT3, 128, NDIR).transpose(1, 2, 0)),
        "dp": np.ascontiguousarray(
            Dp.T.reshape(DT3, 128, NDIR).transpose(1, 2, 0)),
        "w_outproj": _bfl(out_proj_w.T.reshape(DT3, 128, MD)
                          .transpose(1, 0, 2)),
        "ln_g_div": np.ascontiguousarray((ln_g / L).reshape(DT3, 128).T),
        "ln_b": np.ascontiguousarray(ln_b.reshape(DT3, 128).T),
        "w_gr": _bfl(gr_w.T.reshape(DT3, 128, RED).transpose(1, 0, 2)),
        "gr_b": gr_b.reshape(RED, 1).copy(),
        "w_cs": _bfl(cs_w.T),
        "cs_b": np.ascontiguousarray(cs_b.reshape(DT3, 128).T),
        "ones_col": np.ones((128, 1), bf16),
        "xloc": _bfl(_pad_rows(
            np.ascontiguousarray(x[:, :, MD:].transpose(2, 0, 1)), 256)
            .reshape(2, 128, B, L)),
    }

    in_maps = []
    for c in range(NCORES):
        c0 = c * CSL
        sl = slice(c0, c0 + CSL)
        w_pw1 = np.zeros((64, IC), np.float32)
        w_pw1[0:CSL] = pw1_w[sl]
        w_pw1[32:32 + CSL] = pw1_w[IC + c0:IC + c0 + CSL]
        dwd = np.zeros((128, KC, 2, 128), np.float32)
        dwb = np.zeros((128, 2), np.float32)
        bsum = np.zeros((2, 128, CSL), np.float32)
        for bb in range(4):
            row = bb * 32
            for k in range(KC):
                dwd[row + np.arange(CSL), k, :, row + np.arange(CSL)] = \
                    dw_w[sl, k][:, None]
            dwb[row:row + CSL, :] = dw_b[sl][:, None]
            for pt in range(2):
                bsum[pt, row:row + CSL, :] = np.eye(CSL, dtype=np.float32)
        p2bias = np.zeros((128, 2), np.float32)
        if c == 0:
            p2bias[:, 0] = pw2_b[:128]
            p2bias[:IC - 128, 1] = pw2_b[128:]
        m = dict(shared)
        m.update({
            "xm": _bfl(_pad_rows(np.ascontiguousarray(x[c, :, :MD].T), 256)
                       .reshape(2, 128, L)),
            "w_pw1": _bfl(_pad_rows(w_pw1.T, 256).reshape(2, 128, 64)),
            "pw1_b_a": pw1_b[sl].reshape(CSL, 1).copy(),
            "pw1_b_g": pw1_b[IC + c0:IC + c0 + CSL].reshape(CSL, 1).copy(),
            "dw_diag": _bfl(dwd),
            "dw_bias": dwb,
            "bn_g": bn_g[sl].reshape(CSL, 1).copy(),
            "bn_b": bn_b[sl].reshape(CSL, 1).copy(),
            "bsum_mat": bsum,
            "w_pw2": _bfl(pw2_w[:, sl].T),
            "pw2_bias": p2bias,
        })
        in_maps.append(m)
    return in_maps


def assemble(results):
    out = np.zeros((B, L, D_MODEL), np.float32)
    for c in range(NCORES):
        out[c, :, :MD] = results[c]["out_mamba"].reshape(256, L)[:MD].T
    loc = np.zeros((B, 2, 128, L), np.float32)
    for c in range(NCORES):
        loc += results[c]["out_local"]
    out[:, :, MD:] = loc.reshape(B, 256, L)[:, :IC].transpose(0, 2, 1)
    return out


_NC_CACHE = None


def kernel(**inputs):
    global _NC_CACHE
    from concourse import bass_utils
    if _NC_CACHE is None:
        _NC_CACHE = build_nc()
    in_maps = build_in_maps(inputs)
    res = bass_utils.run_bass_kernel_spmd(_NC_CACHE, in_maps,
                                          list(range(NCORES)))
    return assemble(res.results)



# revision 8
# speedup vs baseline: 1.0744x; 1.0744x over previous
"""MixMamba Trainium2 Bass kernel (8-core SPMD).

Sharding:
 - Mamba branch: data-parallel, core = batch element (8 batches, 8 cores).
 - Local conv branch: channel-parallel (24 of 192 channels per core, all
   batches) because training-mode BatchNorm needs cross-batch stats; the
   host sums the per-core partial pw2 outputs during unshard.

Mamba selective scan: for each (direction, d-tile, state n) the recurrence
h_t = exp(-n*delta_t)*h_{t-1} + B_t[n]*delta_t*u_t runs as one hardware
`tensor_tensor_scan` over 1025 elements (1024 steps + 1 zero "reset" pad
column so consecutive uses never leak state).  The decay tensor is built
on the Scalar engine as Exp(scale*delta) with scale=-n (free affine), the
B/C rows are partition-broadcast by DMA, and y = sum_n C_n*h_n accumulates
through a balanced binary tree of bf16 adds on the Vector engine.
"""
import os
import sys
import numpy as np

for _p in ("/opt/trn_rl_repo",):
    if _p not in sys.path and os.path.isdir(_p):
        sys.path.insert(0, _p)

import ml_dtypes

bf16 = ml_dtypes.bfloat16

B, Hh, Ww = 8, 32, 32
L = 1024
D_MODEL = 384
MD = 192
DI = 384
NS = 16
DC = 4
DTR = 12
NDIR = 4
RED = 48
IC = 192
KC = 8
NCORES = 8
CSL = IC // NCORES
DT3 = 3
LP = L + 1
HF = L // 512  # matmul column halves

def build_nc():
    import os as _os
    SKIP_LOCAL = _os.environ.get("K_SKIP_LOCAL") == "1"
    SKIP_SCAN = _os.environ.get("K_SKIP_SCAN") == "1"
    SKIP_BIATTN = _os.environ.get("K_SKIP_BIATTN") == "1"
    NDIRS = int(_os.environ.get("K_NDIRS", NDIR))
    from concourse import bacc, tile, mybir

    f32 = mybir.dt.float32
    bfl = mybir.dt.bfloat16
    AF = mybir.ActivationFunctionType
    OP = mybir.AluOpType
    X = mybir.AxisListType.X

    nc = bacc.Bacc("TRN2", target_bir_lowering=False, debug=False,
                   num_devices=NCORES)

    def din(name, shape, dt=bfl):
        return nc.dram_tensor(name, list(shape), dt, kind="ExternalInput")

    d_xm = din("xm", (2, 128, L))
    d_winp = din("w_inproj", (2, 128, 2 * DI))
    d_cvd = din("conv_diag", (NDIR, 128, DC, DT3, 128))
    d_convb = din("conv_bias", (128, NDIR, DT3), f32)
    d_wxp = din("w_xproj", (128, NDIR, DT3, DTR + 2 * NS))
    d_wdt = din("w_dtproj", (DTR, NDIR, DI))
    d_dtb = din("dt_bias", (128, NDIR, DT3), f32)
    d_dp = din("dp", (128, NDIR, DT3), f32)
    d_wout = din("w_outproj", (128, DT3, MD))
    d_lng = din("ln_g_div", (128, DT3), f32)
    d_lnb = din("ln_b", (128, DT3), f32)
    d_wgr = din("w_gr", (128, DT3, RED))
    d_grb = din("gr_b", (RED, 1), f32)
    d_wcs = din("w_cs", (RED, DI))
    d_csb = din("cs_b", (128, DT3), f32)
    d_ones = din("ones_col", (128, 1))
    d_ident = din("ident", (128, 128))
    d_xloc = din("xloc", (2, 128, B, L))
    d_wpw1 = din("w_pw1", (2, 128, 64))
    d_p1ba = din("pw1_b_a", (CSL, 1), f32)
    d_p1bg = din("pw1_b_g", (CSL, 1), f32)
    d_dwd = din("dw_diag", (128, KC, 2, 128))
    d_dwb = din("dw_bias", (128, 2), f32)
    d_bng = din("bn_g", (CSL, 1), f32)
    d_bnb = din("bn_b", (CSL, 1), f32)
    d_bsum = din("bsum_mat", (2, 128, CSL), f32)
    d_wpw2 = din("w_pw2", (CSL, IC))
    d_p2b = din("pw2_bias", (128, 2), f32)

    d_out_m = nc.dram_tensor("out_mamba", [2, 128, L], f32,
                             kind="ExternalOutput")
    d_out_l = nc.dram_tensor("out_local", [B, 2, 128, L], f32,
                             kind="ExternalOutput")

    with tile.TileContext(nc) as tc, \
            tc.tile_pool(name="w", bufs=1) as wp, \
            tc.tile_pool(name="pers", bufs=1) as pp, \
            tc.tile_pool(name="tr1", bufs=1) as tr1, \
            tc.tile_pool(name="tr2", bufs=2) as tr2, \
            tc.tile_pool(name="dr", bufs=2, space="DRAM") as dr, \
            tc.tile_pool(name="ps", bufs=1, space="PSUM") as ps, \
            tc.tile_pool(name="app", bufs=1, space="PSUM") as app, \
            tc.tile_pool(name="psm", bufs=1, space="PSUM") as psm:

        import itertools as _it
        _cnt = _it.count()

        def mktile(pool, shape, dt, tag):
            return pool.tile(list(shape), dt, tag=tag,
                             name=f"{tag.replace(' ', '')}_{next(_cnt)}")

        def load(dram, shape, dt=bfl, tag=None, pool=wp):
            t = mktile(pool, shape, dt, tag)
            nc.sync.dma_start(t[:], dram[:] if not isinstance(dram, tuple)
                              else dram[0])
            return t

        s_xm = [mktile(wp, (128, L), bfl, f"xm{i}") for i in range(2)]
        s_winp = [mktile(wp, (128, 2 * DI), bfl, f"wi{i}") for i in range(2)]
        for i in range(2):
            nc.sync.dma_start(s_xm[i][:], d_xm[i])
            nc.sync.dma_start(s_winp[i][:], d_winp[i])
        s_wxp = load(d_wxp, (128, NDIR, DT3, DTR + 2 * NS), tag="wxp")
        s_wdt = load(d_wdt, (DTR, NDIR, DI), tag="wdt")
        s_convb = load(d_convb, (128, NDIR, DT3), f32, tag="convb")
        s_dtb = load(d_dtb, (128, NDIR, DT3), f32, tag="dtb")
        s_dp = load(d_dp, (128, NDIR, DT3), f32, tag="dp")
        s_wout = load(d_wout, (128, DT3, MD), tag="wout")
        s_lng = load(d_lng, (128, DT3), f32, tag="lng")
        s_lnb = load(d_lnb, (128, DT3), f32, tag="lnb")
        s_wgr = load(d_wgr, (128, DT3, RED), tag="wgr")
        s_grb = load(d_grb, (RED, 1), f32, tag="grb")
        s_wcs = load(d_wcs, (RED, DI), tag="wcs")
        s_csb = load(d_csb, (128, DT3), f32, tag="csb")
        s_ones = load(d_ones, (128, 1), tag="ones")
        s_ident = load(d_ident, (128, 128), tag="ident")
        s_wpw1 = [mktile(wp, (128, 64), bfl, f"w1{i}") for i in range(2)]
        s_bsum = [mktile(wp, (128, CSL), f32, f"bs{i}") for i in range(2)]
        for i in range(2):
            nc.sync.dma_start(s_wpw1[i][:], d_wpw1[i])
            nc.sync.dma_start(s_bsum[i][:], d_bsum[i])
        s_p1ba = load(d_p1ba, (CSL, 1), f32, tag="p1ba")
        s_p1bg = load(d_p1bg, (CSL, 1), f32, tag="p1bg")
        s_dwd = load(d_dwd, (128, KC, 2, 128), tag="dwd")
        s_dwb = load(d_dwb, (128, 2), f32, tag="dwb")
        s_bng = load(d_bng, (CSL, 1), f32, tag="bng")
        s_bnb = load(d_bnb, (CSL, 1), f32, tag="bnb")
        s_wpw2 = load(d_wpw2, (CSL, IC), tag="wpw2")
        s_p2b = load(d_p2b, (128, 2), f32, tag="p2b")

        epsb = mktile(pp, (128, 1), f32, "epsb")
        nc.gpsimd.memset(epsb[:], 1e-5)

        def mm(out_ap, lhsT, rhs_ap, start, stop, cols=L):
            """matmul split into <=512-column pieces."""
            nh = (cols + 511) // 512
            for h in range(nh):
                c0, c1 = h * 512, min((h + 1) * 512, cols)
                nc.tensor.matmul(out_ap[:, c0:c1], lhsT, rhs_ap[:, c0:c1],
                                 start=start, stop=stop)

        if not SKIP_LOCAL:
            # ======== local branch: pw1 + GLU ========
            hglu = [mktile(pp, (128, L), bfl, f"hglu{i}") for i in range(2)]
            for pt in range(2):
                nc.gpsimd.memset(hglu[pt][:], 0.0)
            for b in range(B):
                pt, row = b // 4, (b % 4) * 32
                p1 = mktile(psm, (64, L), f32, "pml")
                for kt in range(2):
                    xlb = mktile(tr1, (128, L), bfl, "xlb")
                    nc.sync.dma_start(xlb[:], d_xloc[kt, :, b, :])
                    mm(p1[:], s_wpw1[kt][:], xlb[:],
                       start=(kt == 0), stop=(kt == 1))
                sig = mktile(tr1, (CSL, L), bfl, "lsig")
                nc.scalar.activation(sig[:], p1[32:32 + CSL, :], AF.Sigmoid,
                                     bias=s_p1bg[:])
                av = mktile(tr1, (CSL, L), bfl, "lav")
                nc.scalar.activation(av[:], p1[0:CSL, :], AF.Identity,
                                     bias=s_p1ba[:])
                nc.vector.tensor_mul(hglu[pt][row:row + CSL, :], av[:], sig[:])

            # ================= local: depthwise conv + BN stats ========
            hconv = [mktile(pp, (128, L), bfl, f"hconv{i}") for i in range(2)]
            lsum = mktile(pp, (128, 2, 2), f32, "lsum")  # [:, pt, 0]=sum 1=sumsq
            for pt in range(2):
                pc = mktile(psm, (128, L), f32, "pml")
                order = [4, 0, 1, 2, 3, 5, 6, 7]  # shift-0 tap first (start)
                for j, k in enumerate(order):
                    sh = 4 - k
                    o0, i0 = max(0, sh), max(0, -sh)
                    ln = L - abs(sh)
                    for h in range(HF):
                        a0 = max(o0, h * 512)
                        a1 = min(o0 + ln, (h + 1) * 512)
                        if a1 <= a0:
                            continue
                        nc.tensor.matmul(
                            pc[:, a0:a1], s_dwd[:, k, pt, :],
                            hglu[pt][:, a0 - sh:a1 - sh],
                            start=(j == 0), stop=(j == len(order) - 1),
                            skip_group_check=True)
                nc.scalar.activation(hconv[pt][:], pc[:], AF.Identity,
                                     bias=s_dwb[:, pt:pt + 1],
                                     accum_out=lsum[:, pt, 0:1])
                sqd = mktile(tr1, (128, L), bfl, "lsq")
                nc.scalar.activation(sqd[:], hconv[pt][:], AF.Square,
                                     accum_out=lsum[:, pt, 1:2])
            pbn = mktile(psm, (CSL, 2), f32, "psms")
            for kt in range(2):
                nc.tensor.matmul(pbn[:], s_bsum[kt][:], lsum[:, kt, :],
                                 start=(kt == 0), stop=(kt == 1))
            st24 = mktile(pp, (CSL, 6), f32, "st24")  # mu E2 musq var rstd -
            nc.scalar.activation(st24[:, 0:1], pbn[:, 0:1], AF.Copy,
                                 scale=1.0 / (B * L))
            nc.scalar.activation(st24[:, 1:2], pbn[:, 1:2], AF.Copy,
                                 scale=1.0 / (B * L))
            nc.scalar.activation(st24[:, 2:3], st24[:, 0:1], AF.Square)
            nc.vector.tensor_sub(st24[:, 3:4], st24[:, 1:2], st24[:, 2:3])
            nc.scalar.activation(st24[:, 4:5], st24[:, 3:4],
                                 AF.Abs_reciprocal_sqrt, bias=epsb[0:CSL, :])
            sc24 = mktile(pp, (CSL, 2), f32, "sc24")  # scale, bias
            nc.vector.tensor_mul(sc24[:, 0:1], st24[:, 4:5], s_bng[:])
            nc.vector.tensor_mul(sc24[:, 1:2], st24[:, 0:1], sc24[:, 0:1])
            nc.vector.tensor_sub(sc24[:, 1:2], s_bnb[:], sc24[:, 1:2])
            screp = mktile(pp, (128, 2, 2), f32, "screp")  # [:, pt, (scale,bias)]
            nc.gpsimd.memset(screp[:], 0.0)
            for pt in range(2):
                for bb in range(4):
                    nc.sync.dma_start(screp[bb * 32:bb * 32 + CSL, pt, :],
                                      sc24[:, :])

        # ================= mamba: in_proj =================
        xz = [mktile(pp, (128, L), bfl, f"xz{m}") for m in range(6)]
        for m in range(6):
            pxz = mktile(ps, (128, L), f32, "pmm")
            for kt in range(2):
                mm(pxz[:], s_winp[kt][:, m * 128:(m + 1) * 128], s_xm[kt][:],
                   start=(kt == 0), stop=(kt == 1))
            nc.scalar.activation(xz[m][:], pxz[:], AF.Copy)

        # ================= mamba per-direction =================
        acc = mktile(pp, (128, DT3, L), f32, "acc")
        nc.gpsimd.memset(acc[:], 0.0)

        for d in range(NDIRS):
            if d == 3 and not SKIP_LOCAL:
                hbns = [mktile(pp, (128, L), bfl, f"hbns{i}") for i in range(2)]
                for pt in range(2):
                    nc.scalar.activation(hbns[pt][:], hconv[pt][:], AF.Silu,
                                         scale=screp[:, pt, 0:1],
                                         bias=screp[:, pt, 1:2])
                for b in range(B):
                    pt, row = b // 4, (b % 4) * 32
                    stage = mktile(tr1, (CSL, L), bfl, "stage")
                    nc.vector.tensor_copy(stage[:], hbns[pt][row:row + CSL, :])
                    for mt, (c0, mw) in enumerate(((0, 128), (128, IC - 128))):
                        pw = mktile(psm, (128, L), f32, "pml")
                        mm(pw[0:mw, :], s_wpw2[:, c0:c0 + mw], stage[:],
                           start=True, stop=True)
                        ev = mktile(tr1, (128, L), f32, "lev")
                        nc.scalar.activation(ev[0:mw, :], pw[0:mw, :], AF.Identity,
                                             bias=s_p2b[0:mw, mt:mt + 1])
                        nc.sync.dma_start(d_out_l[b, mt, 0:mw, :], ev[0:mw, :])


            cvd_t = mktile(tr1, (128, DC, DT3, 128), bfl, "cvd")
            nc.sync.dma_start(cvd_t[:], d_cvd[d])
            # --- direction-ordered xz ---
            # d0: token order; d1: reversed token order (folded into read
            # APs); d2: materialized v-order; d3: reversed reads of d2 tile.
            if d == 2:
                xzd_t = mktile(tr2, (128, 6, L), bfl, "xzd")
                for m in range(6):
                    nc.vector.tensor_copy(
                        xzd_t[:, m, :].rearrange("p (w h) -> p w h", w=Ww),
                        xz[m][:].rearrange("p (h w) -> p h w", h=Hh)
                        .transpose([0, 2, 1]))
                xzd_prev = xzd_t

            def src_ap(m):
                """Direction-ordered source, forward order (no reversal)."""
                if d <= 1:
                    return xz[m][:]
                return xzd_prev[:, m, :]

            def rhs_win(m, a0, a1, sh):
                """conv rhs window: dir-ordered seq positions [a0-sh, a1-sh)."""
                base = src_ap(m)
                if d in (0, 2):
                    return base[:, a0 - sh:a1 - sh]
                x0, x1 = L - (a1 - sh), L - (a0 - sh)
                return base[:, x0:x1][:, ::-1]

            # --- conv + silu -> u ; silu(z) -> sz ---
            # For d1/d3, sz is kept in FORWARD (d0/d2) order; the gate mul
            # reads part[0] reversed instead.
            u_t = mktile(tr2, (128, DT3, L), bfl, "u")
            du_t = mktile(tr2, (128, DT3, L), bfl, "du")
            sz_t = mktile(tr2, (128, DT3, L), bfl, "sz")
            dl_t = mktile(tr2, (128, DT3, L), bfl, "delta")
            for j in range(DT3):
                pu = mktile(ps, (128, L), f32, "pmm")
                order = [3, 0, 1, 2]
                for jj, k in enumerate(order):
                    sh = 3 - k
                    for h in range(HF):
                        a0 = max(sh, h * 512)
                        a1 = (h + 1) * 512
                        if a1 <= a0:
                            continue
                        nc.tensor.matmul(
                            pu[:, a0:a1], cvd_t[:, k, j, :],
                            rhs_win(j, a0, a1, sh),
                            start=(jj == 0), stop=(jj == len(order) - 1),
                            skip_group_check=True)
                nc.scalar.activation(u_t[:, j, :], pu[:], AF.Silu,
                                     bias=s_convb[:, d, j:j + 1])
                nc.scalar.activation(sz_t[:, j, :], src_ap(j + 3), AF.Silu)
            # --- x_proj ---
            pdbl = mktile(ps, (DTR + 2 * NS, L), f32, "pmm")
            for j in range(DT3):
                mm(pdbl[:], s_wxp[:, d, j, :], u_t[:, j, :],
                   start=(j == 0), stop=(j == DT3 - 1))
            dbl = mktile(tr1, (DTR + 2 * NS, L), bfl, "dbl")
            nc.scalar.activation(dbl[:], pdbl[:], AF.Copy)
            ddbl = mktile(dr, (DTR + 2 * NS, L), bfl, "ddbl")
            nc.sync.dma_start(ddbl[:], dbl[:])
            # --- delta / du ---
            for j in range(DT3):
                pdp = mktile(ps, (128, L), f32, "pmm")
                mm(pdp[:], s_wdt[:, d, j * 128:(j + 1) * 128], dbl[0:DTR, :],
                   start=True, stop=True)
                # softplus(x) = ln(1 + exp(x)); Exp and Ln share a table set
                spt = mktile(tr1, (128, L), bfl, "spt")
                nc.scalar.activation(spt[:], pdp[:], AF.Exp,
                                     bias=s_dtb[:, d, j:j + 1])
                nc.scalar.activation(dl_t[:, j, :], spt[:], AF.Ln, bias=1.0)
            nc.vector.tensor_mul(du_t[:], dl_t[:], u_t[:])

            if not SKIP_SCAN:
                # --- scan; sum over states n accumulates on PE via
                # identity-matmuls into a PSUM tile (no DVE add tree) ---
                got = mktile(tr1, (128, DT3, L), bfl, "got")
                for j in range(DT3):
                    accp = mktile(app, (128, L), f32, "accp")
                    a_t = [mktile(pp, (128, LP), f32, f"a{i}") for i in range(2)]
                    b_t = mktile(pp, (128, LP), bfl, "b0")
                    h_t = mktile(pp, (128, LP), bfl, "h0")
                    for t_ in a_t:
                        nc.gpsimd.memset(t_[:, L:LP], 0.0)
                    nc.gpsimd.memset(b_t[:, L:LP], 0.0)
                    part = [mktile(pp, (128, L), bfl, f"p{lv}") for lv in range(2)]
                    yt = mktile(pp, (128, L), bfl, "yt")
                    for n in range(1, NS + 1):
                        sl = (n - 1) % 2
                        brep = mktile(pp, (128, L), bfl, f"br{sl}")
                        crep = mktile(pp, (128, L), bfl, f"cr{sl}")
                        nc.scalar.activation(a_t[sl][:, 0:L], dl_t[:, j, :],
                                             AF.Exp, scale=-float(n))
                        nc.sync.dma_start(
                            brep[:],
                            ddbl[DTR + n - 1:DTR + n, :].to_broadcast((128, L)))
                        nc.sync.dma_start(
                            crep[:],
                            ddbl[DTR + NS + n - 1:DTR + NS + n, :]
                            .to_broadcast((128, L)))
                        nc.vector.tensor_mul(b_t[:, 0:L], du_t[:, j, :], brep[:])
                        nc.vector.tensor_tensor_scan(h_t[:], a_t[sl][:], b_t[:],
                                                     0.0, OP.mult, OP.add)
                        nc.vector.tensor_mul(part[sl][:], h_t[:, 0:L],
                                             crep[:])
                        for h in range(HF):
                            nc.tensor.matmul(
                                accp[:, h * 512:(h + 1) * 512], s_ident[:],
                                part[sl][:, h * 512:(h + 1) * 512],
                                start=(n == 1), stop=(n == NS),
                                skip_group_check=True)
                    # y = state_sum + u*Dp
                    nc.vector.scalar_tensor_tensor(
                        yt[:], u_t[:, j, :], s_dp[:, d, j:j + 1],
                        accp[:], OP.mult, OP.add)
                    # gate with silu(z), written back to original token order
                    if d == 0:
                        nc.vector.tensor_mul(got[:, j, :], yt[:],
                                             sz_t[:, j, :])
                    elif d == 1:
                        nc.vector.tensor_mul(got[:, j, :], yt[:, ::-1],
                                             sz_t[:, j, :])
                    else:
                        # contiguous gate mul in v-order, then one strided
                        # copy into token order (4.6us strided mul -> 1.7us)
                        gtmp = mktile(pp, (128, L), bfl, "gtmp")
                        if d == 2:
                            nc.vector.tensor_mul(gtmp[:], yt[:],
                                                 sz_t[:, j, :])
                        else:
                            nc.vector.tensor_mul(gtmp[:], yt[:, ::-1],
                                                 sz_t[:, j, :])
                        gv = got[:, j, :] \
                            .rearrange("p (h w) -> p h w", h=Hh) \
                            .transpose([0, 2, 1])
                        nc.vector.tensor_copy(
                            gv, gtmp[:].rearrange("p (w h) -> p w h", w=Ww))

            if not SKIP_BIATTN:
                # --- biattn ---
                sq = mktile(tr1, (128, DT3, L), bfl, "bsqt")
                nc.scalar.activation(sq[:], got[:], AF.Square)
                dstat = mktile(dr, (2, L), bfl, "dstat")
                for src_t, outrow in ((got, 0), (sq, 1)):
                    rowb = mktile(pp, (1, L), bfl, "rowb")
                    for h in range(HF):
                        pbi = mktile(psm, (1, 512), f32, "psm")
                        for j in range(DT3):
                            nc.tensor.matmul(pbi[:], s_ones[:],
                                             src_t[:, j, h * 512:(h + 1) * 512],
                                             start=(j == 0), stop=(j == DT3 - 1))
                        nc.scalar.activation(rowb[:, h * 512:(h + 1) * 512],
                                             pbi[:], AF.Copy, scale=1.0 / DI)
                    nc.sync.dma_start(dstat[outrow:outrow + 1, :], rowb[:])
                # t-partitioned stats: (128, 8) with t = p*8 + f
                sumT = mktile(pp, (128, 2, 8), bfl, "sumT")
                nc.sync.dma_start(sumT[:],
                                  dstat[:].rearrange("r (p f) -> p r f", p=128))
                mu8 = mktile(pp, (128, 8), f32, "mu8")
                ms8 = mktile(pp, (128, 8), f32, "ms8")
                rstd8 = mktile(pp, (128, 8), f32, "rstd8")
                nc.vector.tensor_copy(mu8[:], sumT[:, 0, :])
                nc.scalar.activation(ms8[:], mu8[:], AF.Square)
                nc.vector.tensor_sub(ms8[:], sumT[:, 1, :], ms8[:])
                nc.scalar.activation(rstd8[:], ms8[:],
                                     AF.Abs_reciprocal_sqrt, bias=epsb[:])
                rstdb = mktile(pp, (128, 8), bfl, "rstdb")
                nc.vector.tensor_copy(rstdb[:], rstd8[:])
                drst = mktile(dr, (1, L), bfl, "drst")
                nc.sync.dma_start(
                    drst[:].rearrange("r (p f) -> p (r f)", p=128), rstdb[:])
                rsb = mktile(pp, (128, L), bfl, "rsb")
                nc.sync.dma_start(rsb[:], drst[:].to_broadcast((128, L)))
                # q2 = sum_t mu*rstd
                q2t8 = mktile(pp, (128, 8), bfl, "q2t8")
                nc.vector.tensor_mul(q2t8[:], mu8[:], rstd8[:])
                pq2 = mktile(psm, (1, 8), f32, "psms")
                nc.tensor.matmul(pq2[:], s_ones[:], q2t8[:], start=True, stop=True)
                q2row = mktile(pp, (1, 8), f32, "q2row")
                nc.scalar.activation(q2row[:], pq2[:], AF.Copy)
                q2 = mktile(pp, (1, 1), f32, "q2")
                nc.vector.reduce_sum(q2[:], q2row[:], axis=X)
                dq2 = mktile(dr, (1, 1), f32, "dq2")
                nc.sync.dma_start(dq2[:], q2[:])
                q2r = mktile(pp, (128, 1), f32, "q2r")
                nc.sync.dma_start(q2r[:], dq2[:].to_broadcast((128, 1)))
                t1 = mktile(tr1, (128, DT3, L), bfl, "bsqt")
                q1 = mktile(pp, (128, DT3), f32, "q1")
                gp = mktile(pp, (128, DT3), f32, "gp")
                gpb = mktile(pp, (128, DT3), bfl, "gpb")
                for j in range(DT3):
                    nc.vector.tensor_mul(t1[:, j, :], got[:, j, :], rsb[:])
                    nc.scalar.activation(t1[:, j, :], t1[:, j, :], AF.Copy,
                                         accum_out=q1[:, j:j + 1])
                    nc.vector.tensor_sub(gp[:, j:j + 1], q1[:, j:j + 1], q2r[:])
                    nc.vector.tensor_mul(gp[:, j:j + 1], gp[:, j:j + 1],
                                         s_lng[:, j:j + 1])
                    nc.vector.tensor_add(gp[:, j:j + 1], gp[:, j:j + 1],
                                         s_lnb[:, j:j + 1])
                nc.vector.tensor_copy(gpb[:], gp[:])
                pgr = mktile(psm, (RED, 1), f32, "psms")
                for j in range(DT3):
                    nc.tensor.matmul(pgr[:], s_wgr[:, j, :], gpb[:, j:j + 1],
                                     start=(j == 0), stop=(j == DT3 - 1))
                gg = mktile(pp, (RED, 4), f32, "gg")
                nc.scalar.activation(gg[:, 0:1], pgr[:], AF.Identity,
                                     bias=s_grb[:])
                nc.scalar.activation(gg[:, 1:2], gg[:, 0:1], AF.Erf,
                                     scale=0.7071067811865476)
                nc.vector.tensor_scalar_add(gg[:, 1:2], gg[:, 1:2], 1.0)
                nc.vector.tensor_mul(gg[:, 2:3], gg[:, 0:1], gg[:, 1:2])
                ggb = mktile(pp, (RED, 1), bfl, "ggb")
                nc.scalar.activation(ggb[:], gg[:, 2:3], AF.Copy, scale=0.5)
                cvec = mktile(pp, (128, DT3), f32, "cvec")
                for j in range(DT3):
                    pcs = mktile(psm, (128, 1), f32, "psms")
                    nc.tensor.matmul(pcs[:], s_wcs[:, j * 128:(j + 1) * 128],
                                     ggb[:], start=True, stop=True)
                    nc.scalar.activation(cvec[:, j:j + 1], pcs[:], AF.Sigmoid,
                                         bias=s_csb[:, j:j + 1])
                    nc.vector.scalar_tensor_tensor(
                        acc[:, j, :], got[:, j, :], cvec[:, j:j + 1],
                        acc[:, j, :], OP.mult, OP.add)

        # ---- out_proj ----
        accb = mktile(pp, (128, DT3, L), bfl, "accb")
        nc.vector.tensor_copy(accb[:], acc[:])
        for mt in range(2):
            mw = 128 if mt == 0 else MD - 128
            pmo = mktile(ps, (128, L), f32, "pmm")
            for j in range(DT3):
                mm(pmo[0:mw, :], s_wout[:, j, mt * 128:mt * 128 + mw],
                   accb[:, j, :], start=(j == 0), stop=(j == DT3 - 1))
            evm = mktile(tr1, (128, L), f32, "lev")
            nc.scalar.activation(evm[0:mw, :], pmo[0:mw, :], AF.Copy)
            nc.sync.dma_start(d_out_m[mt, 0:mw, :], evm[0:mw, :])

    nc.compile()
    return nc


# ------------------------------------------------------------- host side
def _pad_rows(a, rows):
    out = np.zeros((rows,) + a.shape[1:], a.dtype)
    out[:a.shape[0]] = a
    return out


def _bfl(a):
    return np.ascontiguousarray(a).astype(bf16)


def build_in_maps(inputs):
    f = lambda k: np.asarray(inputs[k], np.float32)
    x = f("x")
    ipw, conv_w, conv_b = f("in_proj_w"), f("conv_w"), f("conv_b")
    xproj_w, dtproj_w, dtproj_b = f("xproj_w"), f("dtproj_w"), f("dtproj_b")
    Dp, out_proj_w = f("Dp"), f("out_proj_w")
    ln_g, ln_b = f("attn_ln_g"), f("attn_ln_b")
    gr_w, gr_b = f("attn_gr_w"), f("attn_gr_b")
    cs_w, cs_b = f("attn_cs_w"), f("attn_cs_b")
    pw1_w, pw1_b = f("pw1_w"), f("pw1_b")
    dw_w, dw_b = f("dw_w"), f("dw_b")
    bn_g, bn_b = f("bn_g"), f("bn_b")
    pw2_w, pw2_b = f("pw2_w"), f("pw2_b")

    cvd = np.zeros((NDIR, 128, DC, DT3, 128), np.float32)
    r = np.arange(128)
    for d in range(NDIR):
        for k in range(DC):
            for j in range(DT3):
                cvd[d, r, k, j, r] = conv_w[d, j * 128:(j + 1) * 128, k]
    shared = {
        "w_inproj": _bfl(_pad_rows(ipw.T, 256).reshape(2, 128, 2 * DI)),
        "conv_diag": _bfl(cvd),
        "conv_bias": np.ascontiguousarray(
            conv_b.T.reshape(DT3, 128, NDIR).transpose(1, 2, 0)),
        "w_xproj": _bfl(xproj_w.transpose(2, 0, 1).reshape(
            DT3, 128, NDIR, DTR + 2 * NS).transpose(1, 2, 0, 3)),
        "w_dtproj": _bfl(dtproj_w.transpose(2, 0, 1)),
        "dt_bias": np.ascontiguousarray(
            dtproj_b.T.reshape(DT3, 128, NDIR).transpose(1, 2, 0)),
        "dp": np.ascontiguousarray(
            Dp.T.reshape(DT3, 128, NDIR).transpose(1, 2, 0)),
        "w_outproj": _bfl(out_proj_w.T.reshape(DT3, 128, MD)
                          .transpose(1, 0, 2)),
        "ln_g_div": np.ascontiguousarray((ln_g / L).reshape(DT3, 128).T),
        "ln_b": np.ascontiguousarray(ln_b.reshape(DT3, 128).T),
        "w_gr": _bfl(gr_w.T.reshape(DT3, 128, RED).transpose(1, 0, 2)),
        "gr_b": gr_b.reshape(RED, 1).copy(),
        "w_cs": _bfl(cs_w.T),
        "cs_b": np.ascontiguousarray(cs_b.reshape(DT3, 128).T),
        "ones_col": np.ones((128, 1), bf16),
        "ident": np.eye(128, dtype=np.float32).astype(bf16),
        "xloc": _bfl(_pad_rows(
            np.ascontiguousarray(x[:, :, MD:].transpose(2, 0, 1)), 256)
            .reshape(2, 128, B, L)),
    }

    in_maps = []
    for c in range(NCORES):
        c0 = c * CSL
        sl = slice(c0, c0 + CSL)
        w_pw1 = np.zeros((64, IC), np.float32)
        w_pw1[0:CSL] = pw1_w[sl]
        w_pw1[32:32 + CSL] = pw1_w[IC + c0:IC + c0 + CSL]
        dwd = np.zeros((128, KC, 2, 128), np.float32)
        dwb = np.zeros((128, 2), np.float32)
        bsum = np.zeros((2, 128, CSL), np.float32)
        for bb in range(4):
            row = bb * 32
            for k in range(KC):
                dwd[row + np.arange(CSL), k, :, row + np.arange(CSL)] = \
                    dw_w[sl, k][:, None]
            dwb[row:row + CSL, :] = dw_b[sl][:, None]
            for pt in range(2):
                bsum[pt, row:row + CSL, :] = np.eye(CSL, dtype=np.float32)
        p2bias = np.zeros((128, 2), np.float32)
        if c == 0:
            p2bias[:, 0] = pw2_b[:128]
            p2bias[:IC - 128, 1] = pw2_b[128:]
        m = dict(shared)
        m.update({
            "xm": _bfl(_pad_rows(np.ascontiguousarray(x[c, :, :MD].T), 256)
                       .reshape(2, 128, L)),
            "w_pw1": _bfl(_pad_rows(w_pw1.T, 256).reshape(2, 128, 64)),
            "pw1_b_a": pw1_b[sl].reshape(CSL, 1).copy(),
            "pw1_b_g": pw1_b[IC + c0:IC + c0 + CSL].reshape(CSL, 1).copy(),
            "dw_diag": _bfl(dwd),
            "dw_bias": dwb,
            "bn_g": bn_g[sl].reshape(CSL, 1).copy(),
            "bn_b": bn_b[sl].reshape(CSL, 1).copy(),
            "bsum_mat": bsum,
            "w_pw2": _bfl(pw2_w[:, sl].T),
            "pw2_bias": p2bias,
        })
        in_maps.append(m)
    return in_maps


def assemble(results):
    out = np.zeros((B, L, D_MODEL), np.float32)
    for c in range(NCORES):
        out[c, :, :MD] = results[c]["out_mamba"].reshape(256, L)[:MD].T
    loc = np.zeros((B, 2, 128, L), np.float32)
    for c in range(NCORES):
        loc += results[c]["out_local"]
    out[:, :, MD:] = loc.reshape(B, 256, L)[:, :IC].transpose(0, 2, 1)
    return out


_NC_CACHE = None


def kernel(**inputs):
    global _NC_CACHE
    from concourse import bass_utils
    if _NC_CACHE is None:
        _NC_CACHE = build_nc()
    in_maps = build_in_maps(inputs)
    res = bass_utils.run_bass_kernel_spmd(_NC_CACHE, in_maps,
                                          list(range(NCORES)))
    return assemble(res.results)



# revision 14
# speedup vs baseline: 1.1242x; 1.0464x over previous
"""MixMamba Trainium2 Bass kernel (8-core SPMD).

Sharding:
 - Mamba branch: data-parallel, core = batch element (8 batches, 8 cores).
 - Local conv branch: channel-parallel (24 of 192 channels per core, all
   batches) because training-mode BatchNorm needs cross-batch stats; the
   host sums the per-core partial pw2 outputs during unshard.

Mamba selective scan: for each (direction, d-tile, state n) the recurrence
h_t = exp(-n*delta_t)*h_{t-1} + B_t[n]*delta_t*u_t runs as one hardware
`tensor_tensor_scan` over 1025 elements (1024 steps + 1 zero "reset" pad
column so consecutive uses never leak state).  The decay tensor is built
on the Scalar engine as Exp(scale*delta) with scale=-n (free affine), the
B/C rows are partition-broadcast by DMA, and y = sum_n C_n*h_n accumulates
through a balanced binary tree of bf16 adds on the Vector engine.
"""
import os
import sys
import numpy as np

for _p in ("/opt/trn_rl_repo",):
    if _p not in sys.path and os.path.isdir(_p):
        sys.path.insert(0, _p)

import ml_dtypes

bf16 = ml_dtypes.bfloat16

B, Hh, Ww = 8, 32, 32
L = 1024
D_MODEL = 384
MD = 192
DI = 384
NS = 16
DC = 4
DTR = 12
NDIR = 4
RED = 48
IC = 192
KC = 8
NCORES = 8
CSL = IC // NCORES
DT3 = 3
LP = L + 1
HF = L // 512  # matmul column halves

def build_nc():
    import os as _os
    SKIP_LOCAL = _os.environ.get("K_SKIP_LOCAL") == "1"
    SKIP_SCAN = _os.environ.get("K_SKIP_SCAN") == "1"
    SKIP_BIATTN = _os.environ.get("K_SKIP_BIATTN") == "1"
    NDIRS = int(_os.environ.get("K_NDIRS", NDIR))
    from concourse import bacc, tile, mybir

    f32 = mybir.dt.float32
    bfl = mybir.dt.bfloat16
    AF = mybir.ActivationFunctionType
    OP = mybir.AluOpType
    X = mybir.AxisListType.X

    nc = bacc.Bacc("TRN2", target_bir_lowering=False, debug=False,
                   num_devices=NCORES)

    def din(name, shape, dt=bfl):
        return nc.dram_tensor(name, list(shape), dt, kind="ExternalInput")

    d_xm = din("xm", (2, 128, L))
    d_winp = din("w_inproj", (2, 128, 2 * DI))
    d_cvd = din("conv_diag", (NDIR, 128, DC, DT3, 128))
    d_convb = din("conv_bias", (128, NDIR, DT3), f32)
    d_wxp = din("w_xproj", (128, NDIR, DT3, DTR + 2 * NS))
    d_wdt = din("w_dtproj", (DTR, NDIR, DI))
    d_dtb = din("dt_bias", (128, NDIR, DT3), f32)
    d_dp = din("dp", (128, NDIR, DT3), f32)
    d_wout = din("w_outproj", (128, DT3, MD))
    d_lng = din("ln_g_div", (128, DT3), f32)
    d_lnb = din("ln_b", (128, DT3), f32)
    d_wgr = din("w_gr", (128, DT3, RED))
    d_grb = din("gr_b", (RED, 1), f32)
    d_wcs = din("w_cs", (RED, DI))
    d_csb = din("cs_b", (128, DT3), f32)
    d_ones = din("ones_col", (128, 1))
    d_ident = din("ident", (128, 128))
    d_xloc = din("xloc", (2, 128, B, L))
    d_wpw1 = din("w_pw1", (2, 128, 64))
    d_p1ba = din("pw1_b_a", (CSL, 1), f32)
    d_p1bg = din("pw1_b_g", (CSL, 1), f32)
    d_dwd = din("dw_diag", (128, KC, 2, 128))
    d_dwb = din("dw_bias", (128, 2), f32)
    d_bng = din("bn_g", (CSL, 1), f32)
    d_bnb = din("bn_b", (CSL, 1), f32)
    d_bsum = din("bsum_mat", (2, 128, CSL), f32)
    d_wpw2 = din("w_pw2", (CSL, IC))
    d_p2b = din("pw2_bias", (128, 2), f32)

    d_out_m = nc.dram_tensor("out_mamba", [2, 128, L], f32,
                             kind="ExternalOutput")
    d_out_l = nc.dram_tensor("out_local", [B, 2, 128, L], f32,
                             kind="ExternalOutput")

    with tile.TileContext(nc) as tc, \
            tc.tile_pool(name="w", bufs=1) as wp, \
            tc.tile_pool(name="pers", bufs=1) as pp, \
            tc.tile_pool(name="tr1", bufs=1) as tr1, \
            tc.tile_pool(name="tr2", bufs=2) as tr2, \
            tc.tile_pool(name="dr", bufs=2, space="DRAM") as dr, \
            tc.tile_pool(name="xzp", bufs=1) as xzp, \
            tc.tile_pool(name="ps", bufs=1, space="PSUM") as ps, \
            tc.tile_pool(name="app", bufs=1, space="PSUM") as app, \
            tc.tile_pool(name="psm", bufs=1, space="PSUM") as psm:

        import itertools as _it
        _cnt = _it.count()

        def mktile(pool, shape, dt, tag):
            return pool.tile(list(shape), dt, tag=tag,
                             name=f"{tag.replace(' ', '')}_{next(_cnt)}")

        def load(dram, shape, dt=bfl, tag=None, pool=wp):
            t = mktile(pool, shape, dt, tag)
            nc.sync.dma_start(t[:], dram[:] if not isinstance(dram, tuple)
                              else dram[0])
            return t

        s_xm = [mktile(wp, (128, L), bfl, f"xm{i}") for i in range(2)]
        s_winp = [mktile(wp, (128, 2 * DI), bfl, f"wi{i}") for i in range(2)]
        for i in range(2):
            nc.sync.dma_start(s_xm[i][:], d_xm[i])
            nc.sync.dma_start(s_winp[i][:], d_winp[i])
        s_wxp = load(d_wxp, (128, NDIR, DT3, DTR + 2 * NS), tag="wxp")
        s_wdt = load(d_wdt, (DTR, NDIR, DI), tag="wdt")
        s_convb = load(d_convb, (128, NDIR, DT3), f32, tag="convb")
        s_dtb = load(d_dtb, (128, NDIR, DT3), f32, tag="dtb")
        s_dp = load(d_dp, (128, NDIR, DT3), f32, tag="dp")
        s_wout = load(d_wout, (128, DT3, MD), tag="wout")
        s_lng = load(d_lng, (128, DT3), f32, tag="lng")
        s_lnb = load(d_lnb, (128, DT3), f32, tag="lnb")
        s_wgr = load(d_wgr, (128, DT3, RED), tag="wgr")
        s_grb = load(d_grb, (RED, 1), f32, tag="grb")
        s_wcs = load(d_wcs, (RED, DI), tag="wcs")
        s_csb = load(d_csb, (128, DT3), f32, tag="csb")
        s_ones = load(d_ones, (128, 1), tag="ones")
        s_ident = load(d_ident, (128, 128), tag="ident")
        s_wpw1 = [mktile(wp, (128, 64), bfl, f"w1{i}") for i in range(2)]
        s_bsum = [mktile(wp, (128, CSL), f32, f"bs{i}") for i in range(2)]
        for i in range(2):
            nc.sync.dma_start(s_wpw1[i][:], d_wpw1[i])
            nc.sync.dma_start(s_bsum[i][:], d_bsum[i])
        s_p1ba = load(d_p1ba, (CSL, 1), f32, tag="p1ba")
        s_p1bg = load(d_p1bg, (CSL, 1), f32, tag="p1bg")
        s_dwd = load(d_dwd, (128, KC, 2, 128), tag="dwd")
        s_dwb = load(d_dwb, (128, 2), f32, tag="dwb")
        s_bng = load(d_bng, (CSL, 1), f32, tag="bng")
        s_bnb = load(d_bnb, (CSL, 1), f32, tag="bnb")
        s_wpw2 = load(d_wpw2, (CSL, IC), tag="wpw2")
        s_p2b = load(d_p2b, (128, 2), f32, tag="p2b")

        epsb = mktile(pp, (128, 1), f32, "epsb")
        nc.gpsimd.memset(epsb[:], 1e-5)

        def mm(out_ap, lhsT, rhs_ap, start, stop, cols=L):
            """matmul split into <=512-column pieces."""
            nh = (cols + 511) // 512
            for h in range(nh):
                c0, c1 = h * 512, min((h + 1) * 512, cols)
                nc.tensor.matmul(out_ap[:, c0:c1], lhsT, rhs_ap[:, c0:c1],
                                 start=start, stop=stop)

        ls = {}  # local-branch tiles, filled by emit_local_front at d==1

        def emit_local_front():
            # ======== local branch: pw1 + GLU ========
            # (emitted after direction 0 so its DMA/PE/ACT work overlaps the
            # DVE-bound scan phase instead of serializing at kernel start)
            hglu = ls["hglu"] = [mktile(pp, (128, L), bfl, f"hglu{i}")
                                 for i in range(2)]
            for pt in range(2):
                nc.gpsimd.memset(hglu[pt][:], 0.0)
            for b in range(B):
                pt, row = b // 4, (b % 4) * 32
                p1 = mktile(psm, (64, L), f32, "pml")
                for kt in range(2):
                    xlb = mktile(tr1, (128, L), bfl, "xlb")
                    nc.sync.dma_start(xlb[:], d_xloc[kt, :, b, :])
                    mm(p1[:], s_wpw1[kt][:], xlb[:],
                       start=(kt == 0), stop=(kt == 1))
                sig = mktile(tr1, (CSL, L), bfl, "lsig")
                nc.scalar.activation(sig[:], p1[32:32 + CSL, :], AF.Sigmoid,
                                     bias=s_p1bg[:])
                av = mktile(tr1, (CSL, L), bfl, "lav")
                nc.scalar.activation(av[:], p1[0:CSL, :], AF.Identity,
                                     bias=s_p1ba[:])
                nc.gpsimd.tensor_mul(hglu[pt][row:row + CSL, :], av[:], sig[:])

            # ================= local: depthwise conv + BN stats ========
            hconv = ls["hconv"] = [mktile(pp, (128, L), bfl, f"hconv{i}")
                                   for i in range(2)]
            lsum = mktile(pp, (128, 2, 2), f32, "lsum")  # [:, pt, 0]=sum 1=sumsq
            for pt in range(2):
                pc = mktile(psm, (128, L), f32, "pml")
                order = [4, 0, 1, 2, 3, 5, 6, 7]  # shift-0 tap first (start)
                for j, k in enumerate(order):
                    sh = 4 - k
                    o0, i0 = max(0, sh), max(0, -sh)
                    ln = L - abs(sh)
                    for h in range(HF):
                        a0 = max(o0, h * 512)
                        a1 = min(o0 + ln, (h + 1) * 512)
                        if a1 <= a0:
                            continue
                        nc.tensor.matmul(
                            pc[:, a0:a1], s_dwd[:, k, pt, :],
                            hglu[pt][:, a0 - sh:a1 - sh],
                            start=(j == 0), stop=(j == len(order) - 1),
                            skip_group_check=True)
                nc.scalar.activation(hconv[pt][:], pc[:], AF.Identity,
                                     bias=s_dwb[:, pt:pt + 1],
                                     accum_out=lsum[:, pt, 0:1])
                sqd = mktile(tr1, (128, L), bfl, "lsq")
                nc.scalar.activation(sqd[:], hconv[pt][:], AF.Square,
                                     accum_out=lsum[:, pt, 1:2])
            pbn = mktile(psm, (CSL, 2), f32, "psms")
            for kt in range(2):
                nc.tensor.matmul(pbn[:], s_bsum[kt][:], lsum[:, kt, :],
                                 start=(kt == 0), stop=(kt == 1))
            st24 = mktile(pp, (CSL, 6), f32, "st24")  # mu E2 musq var rstd -
            nc.scalar.activation(st24[:, 0:1], pbn[:, 0:1], AF.Copy,
                                 scale=1.0 / (B * L))
            nc.scalar.activation(st24[:, 1:2], pbn[:, 1:2], AF.Copy,
                                 scale=1.0 / (B * L))
            nc.scalar.activation(st24[:, 2:3], st24[:, 0:1], AF.Square)
            nc.vector.tensor_sub(st24[:, 3:4], st24[:, 1:2], st24[:, 2:3])
            nc.scalar.activation(st24[:, 4:5], st24[:, 3:4],
                                 AF.Abs_reciprocal_sqrt, bias=epsb[0:CSL, :])
            sc24 = mktile(pp, (CSL, 2), f32, "sc24")  # scale, bias
            nc.vector.tensor_mul(sc24[:, 0:1], st24[:, 4:5], s_bng[:])
            nc.vector.tensor_mul(sc24[:, 1:2], st24[:, 0:1], sc24[:, 0:1])
            nc.vector.tensor_sub(sc24[:, 1:2], s_bnb[:], sc24[:, 1:2])
            screp = ls["screp"] = mktile(pp, (128, 2, 2), f32, "screp")
            nc.gpsimd.memset(screp[:], 0.0)
            for pt in range(2):
                for bb in range(4):
                    nc.sync.dma_start(screp[bb * 32:bb * 32 + CSL, pt, :],
                                      sc24[:, :])

        # ================= mamba: in_proj =================
        xz = [mktile(pp, (128, L), bfl, f"xz{m}") for m in range(6)]
        for m in range(6):
            pxz = mktile(ps, (128, L), f32, "pmm")
            for kt in range(2):
                mm(pxz[:], s_winp[kt][:, m * 128:(m + 1) * 128], s_xm[kt][:],
                   start=(kt == 0), stop=(kt == 1))
            nc.scalar.activation(xz[m][:], pxz[:], AF.Copy)

        # ================= mamba per-direction =================
        acc = mktile(pp, (128, DT3, L), f32, "acc")
        nc.gpsimd.memset(acc[:], 0.0)

        for d in range(NDIRS):
            if d == 1 and not SKIP_LOCAL:
                emit_local_front()
            if d == 3 and not SKIP_LOCAL:
                hconv, screp = ls["hconv"], ls["screp"]
                hbns = [mktile(pp, (128, L), bfl, f"hglu{i}")
                        for i in range(2)]  # reuses dead hglu buffers
                for pt in range(2):
                    nc.scalar.activation(hbns[pt][:], hconv[pt][:], AF.Silu,
                                         scale=screp[:, pt, 0:1],
                                         bias=screp[:, pt, 1:2])
                for b in range(B):
                    pt, row = b // 4, (b % 4) * 32
                    stage = mktile(tr1, (CSL, L), bfl, "stage")
                    nc.vector.tensor_copy(stage[:], hbns[pt][row:row + CSL, :])
                    for mt, (c0, mw) in enumerate(((0, 128), (128, IC - 128))):
                        pw = mktile(psm, (128, L), f32, "pml")
                        mm(pw[0:mw, :], s_wpw2[:, c0:c0 + mw], stage[:],
                           start=True, stop=True)
                        ev = mktile(tr1, (128, L), f32, "lev")
                        nc.scalar.activation(ev[0:mw, :], pw[0:mw, :], AF.Identity,
                                             bias=s_p2b[0:mw, mt:mt + 1])
                        nc.sync.dma_start(d_out_l[b, mt, 0:mw, :], ev[0:mw, :])


            cvd_t = mktile(tr1, (128, DC, DT3, 128), bfl, "cvd")
            nc.sync.dma_start(cvd_t[:], d_cvd[d])
            # --- direction-ordered xz ---
            # d0: token order; d1: reversed token order (folded into read
            # APs); d2: materialized v-order; d3: reversed reads of d2 tile.
            if d == 2:
                xzd_t = mktile(xzp, (128, 6, L), bfl, "xzd")
                for m in range(6):
                    nc.vector.tensor_copy(
                        xzd_t[:, m, :].rearrange("p (w h) -> p w h", w=Ww),
                        xz[m][:].rearrange("p (h w) -> p h w", h=Hh)
                        .transpose([0, 2, 1]))
                xzd_prev = xzd_t

            def src_ap(m):
                """Direction-ordered source, forward order (no reversal)."""
                if d <= 1:
                    return xz[m][:]
                return xzd_prev[:, m, :]

            def rhs_win(m, a0, a1, sh):
                """conv rhs window: dir-ordered seq positions [a0-sh, a1-sh)."""
                base = src_ap(m)
                if d in (0, 2):
                    return base[:, a0 - sh:a1 - sh]
                x0, x1 = L - (a1 - sh), L - (a0 - sh)
                return base[:, x0:x1][:, ::-1]

            # --- conv + silu -> u ; silu(z) -> sz ---
            # For d1/d3, sz is kept in FORWARD (d0/d2) order; the gate mul
            # reads part[0] reversed instead.
            u_t = mktile(tr2, (128, DT3, L), bfl, "u")
            du_t = mktile(tr2, (128, DT3, L), bfl, "du")
            sz_t = mktile(tr2, (128, DT3, L), bfl, "sz")
            dl_t = mktile(tr2, (128, DT3, L), bfl, "delta")
            for j in range(DT3):
                pu = mktile(ps, (128, L), f32, "pmm")
                order = [3, 0, 1, 2]
                for jj, k in enumerate(order):
                    sh = 3 - k
                    for h in range(HF):
                        a0 = max(sh, h * 512)
                        a1 = (h + 1) * 512
                        if a1 <= a0:
                            continue
                        nc.tensor.matmul(
                            pu[:, a0:a1], cvd_t[:, k, j, :],
                            rhs_win(j, a0, a1, sh),
                            start=(jj == 0), stop=(jj == len(order) - 1),
                            skip_group_check=True)
                nc.scalar.activation(u_t[:, j, :], pu[:], AF.Silu,
                                     bias=s_convb[:, d, j:j + 1])
                nc.scalar.activation(sz_t[:, j, :], src_ap(j + 3), AF.Silu)
            # --- x_proj ---
            pdbl = mktile(ps, (DTR + 2 * NS, L), f32, "pmm")
            for j in range(DT3):
                mm(pdbl[:], s_wxp[:, d, j, :], u_t[:, j, :],
                   start=(j == 0), stop=(j == DT3 - 1))
            dbl = mktile(tr1, (DTR + 2 * NS, L), bfl, "dbl")
            nc.scalar.activation(dbl[:], pdbl[:], AF.Copy)
            ddbl = mktile(dr, (DTR + 2 * NS, L), bfl, "ddbl")
            nc.sync.dma_start(ddbl[:], dbl[:])
            # --- delta / du ---
            for j in range(DT3):
                pdp = mktile(ps, (128, L), f32, "pmm")
                mm(pdp[:], s_wdt[:, d, j * 128:(j + 1) * 128], dbl[0:DTR, :],
                   start=True, stop=True)
                # softplus(x) = ln(1 + exp(x)); Exp and Ln share a table set
                spt = mktile(tr1, (128, L), bfl, "spt")
                nc.scalar.activation(spt[:], pdp[:], AF.Exp,
                                     bias=s_dtb[:, d, j:j + 1])
                nc.scalar.activation(dl_t[:, j, :], spt[:], AF.Ln, bias=1.0)
            nc.vector.tensor_mul(du_t[:], dl_t[:], u_t[:])

            if not SKIP_SCAN:
                # --- scan: states paired (2 per instruction, zero pad column
                # between resets state); state-sum accumulates on PE via
                # identity-matmuls into a PSUM tile (no DVE add tree) ---
                got = mktile(tr1, (128, DT3, L), bfl, "got")
                for j in range(DT3):
                    accp = mktile(app, (128, L), f32, "accp")
                    a_t = [mktile(pp, (128, 2, LP), f32, f"a{i}")
                           for i in range(2)]
                    b_t = mktile(pp, (128, 2, LP), bfl, "b0")
                    h_t = mktile(pp, (128, 2, LP), bfl, "h0")
                    for t_ in a_t:
                        nc.gpsimd.memset(t_[:, :, L:LP], 0.0)
                    nc.gpsimd.memset(b_t[:, :, L:LP], 0.0)
                    part = [mktile(pp, (128, 2, L), bfl, f"p{lv}")
                            for lv in range(2)]
                    yt = mktile(pp, (128, L), bfl, "yt")
                    for np_ in range(NS // 2):  # states 2*np_+1, 2*np_+2
                        sl = np_ % 2
                        brep = mktile(pp, (128, 2, L), bfl, "br")
                        crep = mktile(pp, (128, 2, L), bfl, "cr")
                        for s in range(2):
                            nc.scalar.activation(
                                a_t[sl][:, s, 0:L], dl_t[:, j, :],
                                AF.Exp, scale=-float(2 * np_ + 1 + s))
                        r0 = DTR + 2 * np_
                        for s in range(2):
                            nc.sync.dma_start(
                                brep[:, s, :],
                                ddbl[r0 + s:r0 + s + 1, :]
                                .to_broadcast((128, L)))
                            nc.sync.dma_start(
                                crep[:, s, :],
                                ddbl[r0 + NS + s:r0 + NS + s + 1, :]
                                .to_broadcast((128, L)))
                        for s in range(2):
                            nc.vector.tensor_mul(b_t[:, s, 0:L],
                                                 du_t[:, j, :], brep[:, s, :])
                        nc.vector.tensor_tensor_scan(
                            h_t[:].rearrange("p s l -> p (s l)"),
                            a_t[sl][:].rearrange("p s l -> p (s l)"),
                            b_t[:].rearrange("p s l -> p (s l)"),
                            0.0, OP.mult, OP.add)
                        nc.vector.tensor_mul(part[sl][:, :, 0:L],
                                             h_t[:, :, 0:L], crep[:])
                        for s in range(2):
                            for h in range(HF):
                                nc.tensor.matmul(
                                    accp[:, h * 512:(h + 1) * 512], s_ident[:],
                                    part[sl][:, s, h * 512:(h + 1) * 512],
                                    start=(np_ == 0 and s == 0),
                                    stop=(np_ == NS // 2 - 1 and s == 1),
                                    skip_group_check=True)
                    # y = state_sum + u*Dp
                    nc.vector.scalar_tensor_tensor(
                        yt[:], u_t[:, j, :], s_dp[:, d, j:j + 1],
                        accp[:], OP.mult, OP.add)
                    # gate with silu(z), written back to original token order
                    if d == 0:
                        nc.vector.tensor_mul(got[:, j, :], yt[:],
                                             sz_t[:, j, :])
                    elif d == 1:
                        nc.vector.tensor_mul(got[:, j, :], yt[:, ::-1],
                                             sz_t[:, j, :])
                    else:
                        # contiguous gate mul in v-order, then one strided
                        # copy into token order (4.6us strided mul -> 1.7us)
                        gtmp = mktile(pp, (128, L), bfl, "gtmp")
                        if d == 2:
                            nc.vector.tensor_mul(gtmp[:], yt[:],
                                                 sz_t[:, j, :])
                        else:
                            nc.vector.tensor_mul(gtmp[:], yt[:, ::-1],
                                                 sz_t[:, j, :])
                        gv = got[:, j, :] \
                            .rearrange("p (h w) -> p h w", h=Hh) \
                            .transpose([0, 2, 1])
                        nc.scalar.activation(
                            gv, gtmp[:].rearrange("p (w h) -> p w h", w=Ww),
                            AF.Copy)

            if not SKIP_BIATTN:
                # --- biattn ---
                sq = mktile(tr1, (128, DT3, L), bfl, "bsqt")
                nc.scalar.activation(sq[:], got[:], AF.Square)
                dstat = mktile(dr, (2, L), bfl, "dstat")
                for src_t, outrow in ((got, 0), (sq, 1)):
                    rowb = mktile(pp, (1, L), bfl, "rowb")
                    for h in range(HF):
                        pbi = mktile(psm, (1, 512), f32, "psm")
                        for j in range(DT3):
                            nc.tensor.matmul(pbi[:], s_ones[:],
                                             src_t[:, j, h * 512:(h + 1) * 512],
                                             start=(j == 0), stop=(j == DT3 - 1))
                        nc.scalar.activation(rowb[:, h * 512:(h + 1) * 512],
                                             pbi[:], AF.Copy, scale=1.0 / DI)
                    nc.sync.dma_start(dstat[outrow:outrow + 1, :], rowb[:])
                # t-partitioned stats: (128, 8) with t = p*8 + f
                sumT = mktile(pp, (128, 2, 8), bfl, "sumT")
                nc.sync.dma_start(sumT[:],
                                  dstat[:].rearrange("r (p f) -> p r f", p=128))
                mu8 = mktile(pp, (128, 8), f32, "mu8")
                ms8 = mktile(pp, (128, 8), f32, "ms8")
                rstd8 = mktile(pp, (128, 8), f32, "rstd8")
                nc.vector.tensor_copy(mu8[:], sumT[:, 0, :])
                nc.scalar.activation(ms8[:], mu8[:], AF.Square)
                nc.vector.tensor_sub(ms8[:], sumT[:, 1, :], ms8[:])
                nc.scalar.activation(rstd8[:], ms8[:],
                                     AF.Abs_reciprocal_sqrt, bias=epsb[:])
                rstdb = mktile(pp, (128, 8), bfl, "rstdb")
                nc.vector.tensor_copy(rstdb[:], rstd8[:])
                drst = mktile(dr, (1, L), bfl, "drst")
                nc.sync.dma_start(
                    drst[:].rearrange("r (p f) -> p (r f)", p=128), rstdb[:])
                rsb = mktile(pp, (128, L), bfl, "rsb")
                nc.sync.dma_start(rsb[:], drst[:].to_broadcast((128, L)))
                # q2 = sum_t mu*rstd
                q2t8 = mktile(pp, (128, 8), bfl, "q2t8")
                nc.vector.tensor_mul(q2t8[:], mu8[:], rstd8[:])
                pq2 = mktile(psm, (1, 8), f32, "psms")
                nc.tensor.matmul(pq2[:], s_ones[:], q2t8[:], start=True, stop=True)
                q2row = mktile(pp, (1, 8), f32, "q2row")
                nc.scalar.activation(q2row[:], pq2[:], AF.Copy)
                q2 = mktile(pp, (1, 1), f32, "q2")
                nc.vector.reduce_sum(q2[:], q2row[:], axis=X)
                dq2 = mktile(dr, (1, 1), f32, "dq2")
                nc.sync.dma_start(dq2[:], q2[:])
                q2r = mktile(pp, (128, 1), f32, "q2r")
                nc.sync.dma_start(q2r[:], dq2[:].to_broadcast((128, 1)))
                t1 = mktile(tr1, (128, DT3, L), bfl, "bsqt")
                q1 = mktile(pp, (128, DT3), f32, "q1")
                gp = mktile(pp, (128, DT3), f32, "gp")
                gpb = mktile(pp, (128, DT3), bfl, "gpb")
                for j in range(DT3):
                    nc.vector.tensor_mul(t1[:, j, :], got[:, j, :], rsb[:])
                    nc.scalar.activation(t1[:, j, :], t1[:, j, :], AF.Copy,
                                         accum_out=q1[:, j:j + 1])
                    nc.vector.tensor_sub(gp[:, j:j + 1], q1[:, j:j + 1], q2r[:])
                    nc.vector.tensor_mul(gp[:, j:j + 1], gp[:, j:j + 1],
                                         s_lng[:, j:j + 1])
                    nc.vector.tensor_add(gp[:, j:j + 1], gp[:, j:j + 1],
                                         s_lnb[:, j:j + 1])
                nc.vector.tensor_copy(gpb[:], gp[:])
                pgr = mktile(psm, (RED, 1), f32, "psms")
                for j in range(DT3):
                    nc.tensor.matmul(pgr[:], s_wgr[:, j, :], gpb[:, j:j + 1],
                                     start=(j == 0), stop=(j == DT3 - 1))
                gg = mktile(pp, (RED, 4), f32, "gg")
                nc.scalar.activation(gg[:, 0:1], pgr[:], AF.Identity,
                                     bias=s_grb[:])
                nc.scalar.activation(gg[:, 1:2], gg[:, 0:1], AF.Erf,
                                     scale=0.7071067811865476)
                nc.vector.tensor_scalar_add(gg[:, 1:2], gg[:, 1:2], 1.0)
                nc.vector.tensor_mul(gg[:, 2:3], gg[:, 0:1], gg[:, 1:2])
                ggb = mktile(pp, (RED, 1), bfl, "ggb")
                nc.scalar.activation(ggb[:], gg[:, 2:3], AF.Copy, scale=0.5)
                cvec = mktile(pp, (128, DT3), f32, "cvec")
                for j in range(DT3):
                    pcs = mktile(psm, (128, 1), f32, "psms")
                    nc.tensor.matmul(pcs[:], s_wcs[:, j * 128:(j + 1) * 128],
                                     ggb[:], start=True, stop=True)
                    nc.scalar.activation(cvec[:, j:j + 1], pcs[:], AF.Sigmoid,
                                         bias=s_csb[:, j:j + 1])
                    nc.vector.scalar_tensor_tensor(
                        acc[:, j, :], got[:, j, :], cvec[:, j:j + 1],
                        acc[:, j, :], OP.mult, OP.add)

        # ---- out_proj ----
        accb = mktile(pp, (128, DT3, L), bfl, "accb")
        nc.scalar.activation(accb[:], acc[:], AF.Copy)
        for mt in range(2):
            mw = 128 if mt == 0 else MD - 128
            pmo = mktile(ps, (128, L), f32, "pmm")
            for j in range(DT3):
                mm(pmo[0:mw, :], s_wout[:, j, mt * 128:mt * 128 + mw],
                   accb[:, j, :], start=(j == 0), stop=(j == DT3 - 1))
            evm = mktile(tr1, (128, L), f32, "lev")
            nc.scalar.activation(evm[0:mw, :], pmo[0:mw, :], AF.Copy)
            nc.sync.dma_start(d_out_m[mt, 0:mw, :], evm[0:mw, :])

    nc.compile()
    return nc


# ------------------------------------------------------------- host side
def _pad_rows(a, rows):
    out = np.zeros((rows,) + a.shape[1:], a.dtype)
    out[:a.shape[0]] = a
    return out


def _bfl(a):
    return np.ascontiguousarray(a).astype(bf16)


def build_in_maps(inputs):
    f = lambda k: np.asarray(inputs[k], np.float32)
    x = f("x")
    ipw, conv_w, conv_b = f("in_proj_w"), f("conv_w"), f("conv_b")
    xproj_w, dtproj_w, dtproj_b = f("xproj_w"), f("dtproj_w"), f("dtproj_b")
    Dp, out_proj_w = f("Dp"), f("out_proj_w")
    ln_g, ln_b = f("attn_ln_g"), f("attn_ln_b")
    gr_w, gr_b = f("attn_gr_w"), f("attn_gr_b")
    cs_w, cs_b = f("attn_cs_w"), f("attn_cs_b")
    pw1_w, pw1_b = f("pw1_w"), f("pw1_b")
    dw_w, dw_b = f("dw_w"), f("dw_b")
    bn_g, bn_b = f("bn_g"), f("bn_b")
    pw2_w, pw2_b = f("pw2_w"), f("pw2_b")

    cvd = np.zeros((NDIR, 128, DC, DT3, 128), np.float32)
    r = np.arange(128)
    for d in range(NDIR):
        for k in range(DC):
            for j in range(DT3):
                cvd[d, r, k, j, r] = conv_w[d, j * 128:(j + 1) * 128, k]
    shared = {
        "w_inproj": _bfl(_pad_rows(ipw.T, 256).reshape(2, 128, 2 * DI)),
        "conv_diag": _bfl(cvd),
        "conv_bias": np.ascontiguousarray(
            conv_b.T.reshape(DT3, 128, NDIR).transpose(1, 2, 0)),
        "w_xproj": _bfl(xproj_w.transpose(2, 0, 1).reshape(
            DT3, 128, NDIR, DTR + 2 * NS).transpose(1, 2, 0, 3)),
        "w_dtproj": _bfl(dtproj_w.transpose(2, 0, 1)),
        "dt_bias": np.ascontiguousarray(
            dtproj_b.T.reshape(DT3, 128, NDIR).transpose(1, 2, 0)),
        "dp": np.ascontiguousarray(
            Dp.T.reshape(DT3, 128, NDIR).transpose(1, 2, 0)),
        "w_outproj": _bfl(out_proj_w.T.reshape(DT3, 128, MD)
                          .transpose(1, 0, 2)),
        "ln_g_div": np.ascontiguousarray((ln_g / L).reshape(DT3, 128).T),
        "ln_b": np.ascontiguousarray(ln_b.reshape(DT3, 128).T),
        "w_gr": _bfl(gr_w.T.reshape(DT3, 128, RED).transpose(1, 0, 2)),
        "gr_b": gr_b.reshape(RED, 1).copy(),
        "w_cs": _bfl(cs_w.T),
        "cs_b": np.ascontiguousarray(cs_b.reshape(DT3, 128).T),
        "ones_col": np.ones((128, 1), bf16),
        "ident": np.eye(128, dtype=np.float32).astype(bf16),
        "xloc": _bfl(_pad_rows(
            np.ascontiguousarray(x[:, :, MD:].transpose(2, 0, 1)), 256)
            .reshape(2, 128, B, L)),
    }

    in_maps = []
    for c in range(NCORES):
        c0 = c * CSL
        sl = slice(c0, c0 + CSL)
        w_pw1 = np.zeros((64, IC), np.float32)
        w_pw1[0:CSL] = pw1_w[sl]
        w_pw1[32:32 + CSL] = pw1_w[IC + c0:IC + c0 + CSL]
        dwd = np.zeros((128, KC, 2, 128), np.float32)
        dwb = np.zeros((128, 2), np.float32)
        bsum = np.zeros((2, 128, CSL), np.float32)
        for bb in range(4):
            row = bb * 32
            for k in range(KC):
                dwd[row + np.arange(CSL), k, :, row + np.arange(CSL)] = \
                    dw_w[sl, k][:, None]
            dwb[row:row + CSL, :] = dw_b[sl][:, None]
            for pt in range(2):
                bsum[pt, row:row + CSL, :] = np.eye(CSL, dtype=np.float32)
        p2bias = np.zeros((128, 2), np.float32)
        if c == 0:
            p2bias[:, 0] = pw2_b[:128]
            p2bias[:IC - 128, 1] = pw2_b[128:]
        m = dict(shared)
        m.update({
            "xm": _bfl(_pad_rows(np.ascontiguousarray(x[c, :, :MD].T), 256)
                       .reshape(2, 128, L)),
            "w_pw1": _bfl(_pad_rows(w_pw1.T, 256).reshape(2, 128, 64)),
            "pw1_b_a": pw1_b[sl].reshape(CSL, 1).copy(),
            "pw1_b_g": pw1_b[IC + c0:IC + c0 + CSL].reshape(CSL, 1).copy(),
            "dw_diag": _bfl(dwd),
            "dw_bias": dwb,
            "bn_g": bn_g[sl].reshape(CSL, 1).copy(),
            "bn_b": bn_b[sl].reshape(CSL, 1).copy(),
            "bsum_mat": bsum,
            "w_pw2": _bfl(pw2_w[:, sl].T),
            "pw2_bias": p2bias,
        })
        in_maps.append(m)
    return in_maps


def assemble(results):
    out = np.zeros((B, L, D_MODEL), np.float32)
    for c in range(NCORES):
        out[c, :, :MD] = results[c]["out_mamba"].reshape(256, L)[:MD].T
    loc = np.zeros((B, 2, 128, L), np.float32)
    for c in range(NCORES):
        loc += results[c]["out_local"]
    out[:, :, MD:] = loc.reshape(B, 256, L)[:, :IC].transpose(0, 2, 1)
    return out


_NC_CACHE = None


def kernel(**inputs):
    global _NC_CACHE
    from concourse import bass_utils
    if _NC_CACHE is None:
        _NC_CACHE = build_nc()
    in_maps = build_in_maps(inputs)
    res = bass_utils.run_bass_kernel_spmd(_NC_CACHE, in_maps,
                                          list(range(NCORES)))
    return assemble(res.results)

